# revision 1
# baseline (speedup 1.0000x reference)
"""MAE-ViT forward on 8 trn2 NeuronCores.

Sharding: data-parallel over B=4 samples x 2-way sequence split (256
tokens/core). Feature-major activations on-chip; bf16 matmuls with fp32
accumulation; fp32 LayerNorm/softmax/residual stream. One K + one V
AllGather (bf16) between the two cores of each sample's pair per
attention layer. LayerNorm scales/biases folded into adjacent weights on
the host.
"""

import numpy as np
import ml_dtypes

import concourse.bass as bass
import concourse.bacc as bacc
import concourse.tile as tile
import concourse.mybir as mybir
from concourse.bass_utils import run_bass_kernel_spmd

BF16 = mybir.dt.bfloat16
F32 = mybir.dt.float32
F32R = mybir.dt.float32r
NBF = ml_dtypes.bfloat16
AF = mybir.ActivationFunctionType
ALU = mybir.AluOpType

# Model dims (hardcoded per problem spec)
B, L = 4, 512
T = 256           # tokens per core
D, DD = 768, 512
PD = 1024         # patch dim
N_ENC, N_DEC = 12, 8
ENC_H, DEC_H = 12, 16
ENC_HD, DEC_HD = 64, 32
EPS = 1e-6
MASK_BIAS = -80.0
GROUPS = [[0, 1], [2, 3], [4, 5], [6, 7]]
NO_CC = False  # timeline-sim mode: replace AllGathers with local DMA copies


def _ln_to_z(nc, pools, x_tiles, F, ones_col, ones_row, z_dt=BF16):
    """LayerNorm stats+apply in feature-major layout.

    x_tiles: F fp32 [128, T] tiles (features on partitions).
    Returns F z tiles of dtype z_dt with z = (x - mean) * rstd per token.
    """
    sbuf, psum_big, psum_stat = pools["sbuf"], pools["ps_big"], pools["ps_av"]
    Dv = F * 128
    # per-token sums of x and x^2 via PE ones-reduction on bf16 casts
    # (elementwise bf16 rounding averages out over 768 terms; accumulate fp32)
    ps_sum = psum_stat.tile([1, T], F32, tag="av", name="stat")
    ps_sq = psum_stat.tile([1, T], F32, tag="av", name="stat")
    sq_pool = pools["sq"]
    xb_tiles, sq_tiles = [], []
    for k in range(F):
        xb = sq_pool.tile([128, T], BF16, tag="xb", name="xb")
        nc.vector.tensor_copy(xb, x_tiles[k])
        sq = sq_pool.tile([128, T], BF16, tag="sq", name="sq")
        nc.vector.tensor_mul(sq, xb, xb)
        xb_tiles.append(xb)
        sq_tiles.append(sq)
    for k in range(F):
        nc.tensor.matmul(ps_sum, ones_col[:, 0:1], xb_tiles[k],
                         start=(k == 0), stop=(k == F - 1))
    for k in range(F):
        nc.tensor.matmul(ps_sq, ones_col[:, 0:1], sq_tiles[k],
                         start=(k == 0), stop=(k == F - 1))
    mean = sbuf.tile([1, T], F32, tag="ln_mean", name="ln_mean")
    nc.vector.tensor_scalar_mul(mean, ps_sum, 1.0 / Dv)
    m2 = sbuf.tile([1, T], F32, tag="ln_m2", name="ln_m2")
    nc.vector.tensor_mul(m2, mean, mean)
    var = sbuf.tile([1, T], F32, tag="ln_var", name="ln_var")
    # var = ps_sq/D - mean^2
    nc.vector.scalar_tensor_tensor(var, ps_sq, 1.0 / Dv, m2, ALU.mult, ALU.subtract)
    sd = sbuf.tile([1, T], F32, tag="ln_sd", name="ln_sd")
    nc.scalar.activation(sd, var, AF.Sqrt, bias=pools["eps"])
    rstd = sbuf.tile([1, T], F32, tag="ln_rstd", name="ln_rstd")
    nc.vector.reciprocal(rstd, sd)
    # broadcast mean/rstd across partitions via exact fp32 K=1 outer product
    mb = psum_big.tile([128, T], F32, tag="big", name="big")
    nc.tensor.matmul(mb, ones_row[0:1, :], mean, start=True, stop=True)
    rb = psum_big.tile([128, T], F32, tag="big", name="big")
    nc.tensor.matmul(rb, ones_row[0:1, :], rstd, start=True, stop=True)
    z_tiles = []
    for k in range(F):
        t = sq_pool.tile([128, T], F32, tag="lnt", name="lnt")
        nc.vector.tensor_sub(t, x_tiles[k], mb)
        z = pools["z"].tile([128, T], z_dt, tag=f"z{k}", name=f"z{k}")
        nc.vector.tensor_mul(z, t, rb)
        z_tiles.append(z)
    return z_tiles


def _attention(nc, pools, z_tiles, F, n_heads, hd, wqk, bqk, wv,
               mbias_sb, cc, ones_row, layer_tag):
    """Full attention for one layer. Returns attn output tiles (fm, bf16)."""
    sbuf, aexp = pools["sbuf"], pools["aexp"]
    ps_big, ps_s, ps_av = pools["ps_big"], pools["ps_s"], pools["ps_av"]
    ones_bf = pools["ones_bf"]
    Dm = F * 128                  # model dim
    KT = L // 128                 # 4 k-tiles over full sequence
    hpt = 128 // hd               # heads per 128-row tile
    scale = 1.0 / np.sqrt(hd)
    dram = cc["dram"]

    # --- K feature-major [Dm, T], written to cc-in; AllGather ---
    k_cc_in = dram.tile([F, 128, T], BF16, tag="k_cc_in", name="k_cc_in")
    k_cc_out = dram.tile([2, F, 128, T], BF16, tag="k_cc_out", name="k_cc_out")
    for m in range(F):
        ps = ps_big.tile([128, T], F32, tag="big", name="big")
        for k in range(F):
            nc.tensor.matmul(ps, wqk[:, k, Dm + 128 * m:Dm + 128 * (m + 1)],
                             z_tiles[k], start=(k == 0), stop=(k == F - 1))
        kl = sbuf.tile([128, T], BF16, tag=f"kloc{m}", name=f"kloc{m}")
        nc.scalar.activation(kl, ps, AF.Identity, bias=bqk[:, F + m:F + m + 1])
        nc.sync.dma_start(k_cc_in[m], kl)
    if NO_CC:
        nc.sync.dma_start(k_cc_out[0], k_cc_in[:])
        nc.sync.dma_start(k_cc_out[1], k_cc_in[:])
    else:
        nc.gpsimd.collective_compute(
            "AllGather", ALU.bypass, replica_groups=GROUPS,
            ins=[k_cc_in[:].opt()], outs=[k_cc_out[:].opt()])

    # --- V token-major [T, Dm] -> cc-in; AllGather ---
    v_cc_in = dram.tile([2, 128, Dm], BF16, tag="v_cc_in", name="v_cc_in")
    v_cc_out = dram.tile([2, 2, 128, Dm], BF16, tag="v_cc_out", name="v_cc_out")
    NV = min(Dm // 2, 512)
    for t in range(2):
        vl = sbuf.tile([128, Dm], BF16, tag="vloc", name="vloc")
        for n in range(Dm // NV):
            ps = ps_big.tile([128, NV], F32, tag="big", name="big")
            for k in range(F):
                nc.tensor.matmul(ps, z_tiles[k][:, 128 * t:128 * (t + 1)],
                                 wv[:, k, NV * n:NV * (n + 1)],
                                 start=(k == 0), stop=(k == F - 1))
            nc.vector.tensor_copy(vl[:, NV * n:NV * (n + 1)], ps)
        nc.sync.dma_start(v_cc_in[t], vl)
    if NO_CC:
        nc.sync.dma_start(v_cc_out[0], v_cc_in[:])
        nc.sync.dma_start(v_cc_out[1], v_cc_in[:])
    else:
        nc.gpsimd.collective_compute(
            "AllGather", ALU.bypass, replica_groups=GROUPS,
            ins=[v_cc_in[:].opt()], outs=[v_cc_out[:].opt()])

    # --- Q feature-major (overlaps the collectives) ---
    q_sb = []
    for m in range(F):
        ps = ps_big.tile([128, T], F32, tag="big", name="big")
        for k in range(F):
            nc.tensor.matmul(ps, wqk[:, k, 128 * m:128 * (m + 1)],
                             z_tiles[k], start=(k == 0), stop=(k == F - 1))
        q = sbuf.tile([128, T], BF16, tag=f"q{m}", name=f"q{m}")
        nc.scalar.activation(q, ps, AF.Identity, bias=bqk[:, m:m + 1])
        q_sb.append(q)

    # --- reassemble K_full [Dm, 512] and V_full [KT][128, nh*(hd+1)] ---
    k_full = []
    for m in range(F):
        kf = sbuf.tile([128, L], BF16, tag=f"kfull{m}", name=f"kfull{m}")
        nc.sync.dma_start(kf[:, 0:T], k_cc_out[0, m])
        nc.sync.dma_start(kf[:, T:L], k_cc_out[1, m])
        k_full.append(kf)
    v_full = []
    for kt in range(KT):
        vf = sbuf.tile([128, n_heads, hd + 1], BF16, tag=f"vfull{kt}",
                       name=f"vfull{kt}")
        nc.vector.memset(vf[:, :, hd:hd + 1], 1.0)
        nc.sync.dma_start(
            vf[:, :, 0:hd],
            v_cc_out[kt // 2, kt % 2].rearrange("p (h d) -> p h d", h=n_heads))
        v_full.append(vf)

    # --- per-head scores / no-max softmax / AV / normalize ---
    attn = []
    for m in range(F):
        a = sbuf.tile([128, T], BF16, tag=f"attn{m}", name=f"attn{m}")
        attn.append(a)
    for ft in range(F):
        uv = aexp.tile([128, T], F32, tag="uv", name="uv")
        for j in range(hpt):
            h = ft * hpt + j
            ro = j * hd
            av = ps_av.tile([hd + 1, T], F32, tag="av", name="av")
            for kt in range(KT):
                s = ps_s.tile([128, T], F32, tag="s", name="s")
                nc.tensor.matmul(s, k_full[ft][ro:ro + hd, 128 * kt:128 * (kt + 1)],
                                 q_sb[ft][ro:ro + hd, :], start=True, stop=True,
                                 tile_position=(ro, 0))
                a_sb = aexp.tile([128, T], BF16, tag="a_exp", name="a_exp")
                if mbias_sb is not None:
                    nc.scalar.activation(a_sb, s, AF.Exp, scale=scale,
                                         bias=mbias_sb[:, kt:kt + 1])
                else:
                    nc.scalar.activation(a_sb, s, AF.Exp, scale=scale)
                nc.tensor.matmul(av, v_full[kt][:, h, :], a_sb,
                                 start=(kt == 0), stop=(kt == KT - 1))
            # stage unnormalized AV in SBUF, then normalize by the
            # ones-row denominator via a per-head broadcast outer product
            nc.scalar.activation(uv[ro:ro + hd, :], av[0:hd, :], AF.Identity)
            rcp = aexp.tile([1, T], F32, tag="rcp", name="rcp")
            nc.vector.reciprocal(rcp, av[hd:hd + 1, :])
            rcb = aexp.tile([1, T], BF16, tag="rcb", name="rcb")
            nc.vector.tensor_copy(rcb, rcp)
            bc = ps_s.tile([128, T], F32, tag="s", name="s")
            nc.tensor.matmul(bc[0:hd, :], ones_bf[0:1, 0:hd], rcb,
                             start=True, stop=True)
            nc.vector.tensor_mul(attn[ft][ro:ro + hd, :], uv[ro:ro + hd, :],
                                 bc[0:hd, :])
    return attn


def _matmul_fm(nc, pools, w, in_tiles, Fk, Fm, moff=0):
    """yield (m, psum) with psum = sum_k w[:,k,(m-moff)-slice]^T @ in_tiles[k]."""
    for m in range(Fm):
        ps = pools["ps_big"].tile([128, T], F32, tag="big", name="big")
        for k in range(Fk):
            nc.tensor.matmul(ps, w[:, k, 128 * (m - moff):128 * (m - moff + 1)],
                             in_tiles[k], start=(k == 0), stop=(k == Fk - 1))
        yield m, ps


def _layer(nc, pools, cc, x_tiles, F, n_heads, hd, waps, ones_col, ones_row,
           mbias_sb, wpools, tagp):
    """One transformer block (attn + MLP) updating x_tiles in place."""
    wq_pool, wv_pool, wp_pool, w1_pool, w2_pool, bias_pool = wpools
    (a_wqk, a_bqk, a_wv, a_wpr, a_bpr, a_wf1, a_bf1, a_wf2, a_bf2) = waps
    Dm = F * 128
    F1 = a_wf1.shape[-1] // 128   # hidden tiles (24 enc / 16 dec)

    wqk = wq_pool.tile([128, F, 2 * Dm], BF16, tag="wqk", name="wqk")
    nc.sync.dma_start(wqk, a_wqk)
    wv = wv_pool.tile([128, F, Dm], BF16, tag="wv", name="wv")
    nc.sync.dma_start(wv, a_wv)
    wpr = wp_pool.tile([128, F, Dm], BF16, tag="wpr", name="wpr")
    nc.sync.dma_start(wpr, a_wpr)
    bqk = bias_pool.tile([128, 2 * F], F32, tag="bqk", name="bqk")
    nc.sync.dma_start(bqk, a_bqk)
    bpr = bias_pool.tile([128, F], F32, tag="bpr", name="bpr")
    nc.sync.dma_start(bpr, a_bpr)
    bf1 = bias_pool.tile([128, F1], F32, tag="bf1", name="bf1")
    nc.sync.dma_start(bf1, a_bf1)
    bf2 = bias_pool.tile([128, F], F32, tag="bf2", name="bf2")
    nc.sync.dma_start(bf2, a_bf2)

    z = _ln_to_z(nc, pools, x_tiles, F, ones_col, ones_row)
    attn = _attention(nc, pools, z, F, n_heads, hd, wqk, bqk, wv,
                      mbias_sb, cc, ones_row, tagp)
    for m, ps in _matmul_fm(nc, pools, wpr, attn, F, F):
        nc.vector.scalar_tensor_tensor(x_tiles[m], ps, bpr[:, m:m + 1],
                                       x_tiles[m], ALU.add, ALU.add)
    z2 = _ln_to_z(nc, pools, x_tiles, F, ones_col, ones_row)

    # f1 streamed in 4 column-blocks of F1/4 m-tiles each
    hmid = []
    mpb = F1 // 4
    for b in range(4):
        w1b = w1_pool.tile([128, F, 128 * mpb], BF16, tag="wf1", name="wf1")
        nc.sync.dma_start(w1b, a_wf1[:, :, 128 * mpb * b:128 * mpb * (b + 1)])
        for m, ps in _matmul_fm(nc, pools, w1b, z2, F, mpb):
            mg = b * mpb + m
            hm = pools["h"].tile([128, T], BF16, tag=f"hm{mg}", name=f"hm{mg}")
            nc.scalar.activation(hm, ps, AF.Gelu, bias=bf1[:, mg:mg + 1])
            hmid.append(hm)
    # f2 streamed in column-blocks of 2 m-tiles
    for b in range(F // 2):
        w2b = w2_pool.tile([128, F1, 256], BF16, tag="wf2", name="wf2")
        nc.sync.dma_start(w2b, a_wf2[:, :, 256 * b:256 * (b + 1)])
        for mm, ps in _matmul_fm(nc, pools, w2b, hmid, F1, 2):
            m = 2 * b + mm
            nc.vector.scalar_tensor_tensor(x_tiles[m], ps, bf2[:, m:m + 1],
                                           x_tiles[m], ALU.add, ALU.add)


def build_program(n_enc=N_ENC, n_dec=N_DEC, no_cc=False):
    global NO_CC
    NO_CC = no_cc
    nc = bacc.Bacc("TRN2", target_bir_lowering=False, debug=False, num_devices=8)

    def inp(name, shape, dt=BF16):
        return nc.dram_tensor(name, shape, dt, kind="ExternalInput").ap()

    # --- inputs (per-core) ---
    patches_t = inp("patches_t", [128, 8, T])
    posf_t = inp("posf_t", [5, T])
    w_pe = inp("w_pe", [128, 8, D])
    b_embed = inp("b_embed", [128, 6], F32)
    w_pos1 = inp("w_pos1", [5, 384])
    b_pos1 = inp("b_pos1", [128, 3], F32)
    w_pos2 = inp("w_pos2", [128, 3, D])
    e_wqk = inp("e_wqk", [n_enc, 128, 6, 1536])
    e_bqk = inp("e_bqk", [n_enc, 128, 12], F32)
    e_wv = inp("e_wv", [n_enc, 128, 6, D])
    e_wpr = inp("e_wpr", [n_enc, 128, 6, D])
    e_bpr = inp("e_bpr", [n_enc, 128, 6], F32)
    e_wf1 = inp("e_wf1", [n_enc, 128, 6, 3072])
    e_bf1 = inp("e_bf1", [n_enc, 128, 24], F32)
    e_wf2 = inp("e_wf2", [n_enc, 128, 24, D])
    e_bf2 = inp("e_bf2", [n_enc, 128, 6], F32)
    mbias = inp("mbias", [128, 4], F32)
    vis = inp("vis", [128, T], F32)
    enw = inp("enw", [128, 6], F32)
    enb = inp("enb", [128, 6], F32)
    w_de = inp("w_de", [128, 6, DD])
    b_de = inp("b_de", [128, 4], F32)
    mtk = inp("mtk", [128, 4, T], F32)
    w_dpos1 = inp("w_dpos1", [5, 256])
    b_dpos1 = inp("b_dpos1", [128, 2], F32)
    w_dpos2 = inp("w_dpos2", [128, 2, DD])
    b_dpos2 = inp("b_dpos2", [128, 4], F32)
    d_wqk = inp("d_wqk", [n_dec, 128, 4, 1024])
    d_bqk = inp("d_bqk", [n_dec, 128, 8], F32)
    d_wv = inp("d_wv", [n_dec, 128, 4, DD])
    d_wpr = inp("d_wpr", [n_dec, 128, 4, DD])
    d_bpr = inp("d_bpr", [n_dec, 128, 4], F32)
    d_wf1 = inp("d_wf1", [n_dec, 128, 4, 2048])
    d_bf1 = inp("d_bf1", [n_dec, 128, 16], F32)
    d_wf2 = inp("d_wf2", [n_dec, 128, 16, DD])
    d_bf2 = inp("d_bf2", [n_dec, 128, 4], F32)
    w_hi = inp("w_hi", [128, 4, PD])
    w_hn = inp("w_hn", [128, 4, PD])
    b_hi = inp("b_hi", [1, PD], F32)
    b_hn = inp("b_hn", [1, PD], F32)
    ind_e_in = inp("ind_e", [2, 128])
    ind_d_in = inp("ind_d", [4, 128])
    out_i = nc.dram_tensor("out_i", [T, PD], F32, kind="ExternalOutput").ap()
    out_n = nc.dram_tensor("out_n", [T, PD], F32, kind="ExternalOutput").ap()

    from contextlib import ExitStack
    with tile.TileContext(nc) as tc, ExitStack() as es:
        sbuf = es.enter_context(tc.tile_pool(name="sbuf", bufs=1))
        aexp = es.enter_context(tc.tile_pool(name="aexp", bufs=4))
        consts = es.enter_context(tc.tile_pool(name="consts", bufs=1))
        xpool = es.enter_context(tc.tile_pool(name="x", bufs=1))
        zpool = es.enter_context(tc.tile_pool(name="z", bufs=2))
        sqpool = es.enter_context(tc.tile_pool(name="sq", bufs=3))
        hpool = es.enter_context(tc.tile_pool(name="h", bufs=1))
        wq_pool = es.enter_context(tc.tile_pool(name="wq", bufs=2))
        wv_pool = es.enter_context(tc.tile_pool(name="wv", bufs=1))
        wp_pool = es.enter_context(tc.tile_pool(name="wp", bufs=1))
        w1_pool = es.enter_context(tc.tile_pool(name="w1", bufs=2))
        w2_pool = es.enter_context(tc.tile_pool(name="w2", bufs=2))
        bias_pool = es.enter_context(tc.tile_pool(name="bias", bufs=2))
        ps_big = es.enter_context(tc.tile_pool(name="ps_big", bufs=4, space="PSUM"))
        ps_s = es.enter_context(tc.tile_pool(name="ps_s", bufs=2, space="PSUM"))
        ps_av = es.enter_context(tc.tile_pool(name="ps_av", bufs=2, space="PSUM"))
        dram = es.enter_context(tc.tile_pool(name="dram", bufs=2, space="DRAM"))

        pools = dict(sbuf=sbuf, aexp=aexp, z=zpool, sq=sqpool, h=hpool,
                     ps_big=ps_big, ps_s=ps_s, ps_av=ps_av)
        cc = dict(dram=dram)
        wpools = (wq_pool, wv_pool, wp_pool, w1_pool, w2_pool, bias_pool)

        ones_col = consts.tile([128, 1], BF16)
        nc.vector.memset(ones_col, 1.0)
        ones_row = consts.tile([1, 128], F32)
        nc.vector.memset(ones_row, 1.0)
        ones_bf = consts.tile([1, 128], BF16)
        nc.vector.memset(ones_bf, 1.0)
        eps_sb = consts.tile([1, 1], F32)
        nc.vector.memset(eps_sb, EPS)
        pools["eps"] = eps_sb
        pools["ones_bf"] = ones_bf

        mbias_sb = consts.tile([128, 4], F32)
        nc.sync.dma_start(mbias_sb, mbias)
        vis_sb = consts.tile([128, T], F32)
        nc.sync.dma_start(vis_sb, vis)
        pf = consts.tile([5, T], BF16)
        nc.sync.dma_start(pf, posf_t)

        # ===== embedding (scoped pool, released after) =====
        embed_pool = tc.alloc_tile_pool(name="embed", bufs=1)
        wp1 = embed_pool.tile([5, 384], BF16)
        nc.sync.dma_start(wp1, w_pos1)
        wp2 = embed_pool.tile([128, 3, D], BF16)
        nc.sync.dma_start(wp2, w_pos2)
        bp1 = embed_pool.tile([128, 3], F32)
        nc.sync.dma_start(bp1, b_pos1)
        pt = embed_pool.tile([128, 8, T], BF16)
        nc.sync.dma_start(pt, patches_t)
        wpe = embed_pool.tile([128, 8, D], BF16)
        nc.sync.dma_start(wpe, w_pe)
        bemb = embed_pool.tile([128, 6], F32)
        nc.sync.dma_start(bemb, b_embed)

        h1 = []
        for m in range(3):
            ps = ps_big.tile([128, T], F32, tag="big", name="big")
            nc.tensor.matmul(ps, wp1[:, 128 * m:128 * (m + 1)], pf,
                             start=True, stop=True)
            t = embed_pool.tile([128, T], BF16, tag=f"h1_{m}", name=f"h1_{m}")
            nc.scalar.activation(t, ps, AF.Gelu, bias=bp1[:, m:m + 1])
            h1.append(t)

        x_tiles = [xpool.tile([128, T], F32, tag=f"x{k}", name=f"x{k}")
                   for k in range(6)]
        for m in range(6):
            ps = ps_big.tile([128, T], F32, tag="big", name="big")
            for k in range(8):
                nc.tensor.matmul(ps, wpe[:, k, 128 * m:128 * (m + 1)],
                                 pt[:, k, :], start=(k == 0), stop=False)
            for k in range(3):
                nc.tensor.matmul(ps, wp2[:, k, 128 * m:128 * (m + 1)],
                                 h1[k], start=False, stop=(k == 2))
            nc.scalar.activation(x_tiles[m], ps, AF.Identity, bias=bemb[:, m:m + 1])
        embed_pool.release()

        # ===== encoder =====
        for i in range(n_enc):
            waps = (e_wqk[i], e_bqk[i], e_wv[i], e_wpr[i], e_bpr[i],
                    e_wf1[i], e_bf1[i], e_wf2[i], e_bf2[i])
            _layer(nc, pools, cc, x_tiles, 6, ENC_H, ENC_HD, waps,
                   ones_col, ones_row, mbias_sb, wpools, f"e{i}")

        # ===== bridge: enc norm + mask + decoder embed (scoped pool) =====
        bridge = tc.alloc_tile_pool(name="bridge", bufs=1)
        enw_sb = bridge.tile([128, 6], F32)
        nc.sync.dma_start(enw_sb, enw)
        enb_sb = bridge.tile([128, 6], F32)
        nc.sync.dma_start(enb_sb, enb)
        ze = _ln_to_z(nc, pools, x_tiles, 6, ones_col, ones_row, z_dt=F32)
        enc_sb = []
        for k in range(6):
            t = bridge.tile([128, T], F32, tag=f"enc_t{k}", name=f"enc_t{k}")
            nc.scalar.activation(t, ze[k], AF.Identity, bias=enb_sb[:, k:k + 1],
                                 scale=enw_sb[:, k:k + 1])
            e = bridge.tile([128, T], BF16, tag=f"enc{k}", name=f"enc{k}")
            nc.vector.tensor_mul(e, t, vis_sb)
            enc_sb.append(e)

        wde = bridge.tile([128, 6, DD], BF16)
        nc.sync.dma_start(wde, w_de)
        bde = bridge.tile([128, 4], F32)
        nc.sync.dma_start(bde, b_de)
        mtk_sb = bridge.tile([128, 4, T], F32)
        nc.sync.dma_start(mtk_sb, mtk)
        wd1 = bridge.tile([5, 256], BF16)
        nc.sync.dma_start(wd1, w_dpos1)
        bd1 = bridge.tile([128, 2], F32)
        nc.sync.dma_start(bd1, b_dpos1)
        wd2 = bridge.tile([128, 2, DD], BF16)
        nc.sync.dma_start(wd2, w_dpos2)
        bd2 = bridge.tile([128, 4], F32)
        nc.sync.dma_start(bd2, b_dpos2)

        h1d = []
        for m in range(2):
            ps = ps_big.tile([128, T], F32, tag="big", name="big")
            nc.tensor.matmul(ps, wd1[:, 128 * m:128 * (m + 1)], pf,
                             start=True, stop=True)
            t = bridge.tile([128, T], BF16, tag=f"h1d_{m}", name=f"h1d_{m}")
            nc.scalar.activation(t, ps, AF.Gelu, bias=bd1[:, m:m + 1])
            h1d.append(t)

        xd_tiles = [xpool.tile([128, T], F32, tag=f"xd{k}", name=f"xd{k}")
                    for k in range(4)]
        for m in range(4):
            ps = ps_big.tile([128, T], F32, tag="big", name="big")
            for k in range(6):
                nc.tensor.matmul(ps, wde[:, k, 128 * m:128 * (m + 1)],
                                 enc_sb[k], start=(k == 0), stop=(k == 5))
            t1 = bridge.tile([128, T], F32, tag="dec_t1", name="dec_t1")
            nc.scalar.activation(t1, ps, AF.Identity, bias=bde[:, m:m + 1])
            nc.vector.tensor_mul(t1, t1, vis_sb)
            nc.vector.tensor_add(t1, t1, mtk_sb[:, m, :])
            ps2 = ps_big.tile([128, T], F32, tag="big", name="big")
            for k in range(2):
                nc.tensor.matmul(ps2, wd2[:, k, 128 * m:128 * (m + 1)],
                                 h1d[k], start=(k == 0), stop=(k == 1))
            t2 = bridge.tile([128, T], F32, tag="dec_t2", name="dec_t2")
            nc.scalar.activation(t2, ps2, AF.Identity, bias=bd2[:, m:m + 1])
            nc.vector.tensor_add(xd_tiles[m], t1, t2)
        bridge.release()

        # ===== decoder =====
        for i in range(n_dec):
            waps = (d_wqk[i], d_bqk[i], d_wv[i], d_wpr[i], d_bpr[i],
                    d_wf1[i], d_bf1[i], d_wf2[i], d_bf2[i])
            _layer(nc, pools, cc, xd_tiles, 4, DEC_H, DEC_HD, waps,
                   ones_col, ones_row, None, wpools, f"d{i}")

        # ===== final norm + heads (token-major output) =====
        tail = tc.alloc_tile_pool(name="tail", bufs=1)
        zf = _ln_to_z(nc, pools, xd_tiles, 4, ones_col, ones_row)
        whi = tail.tile([128, 4, PD], BF16)
        nc.sync.dma_start(whi, w_hi)
        whn = tail.tile([128, 4, PD], BF16)
        nc.sync.dma_start(whn, w_hn)
        bhi = tail.tile([128, PD], F32)
        nc.sync.dma_start(bhi, b_hi.to_broadcast([128, PD]))
        bhn = tail.tile([128, PD], F32)
        nc.sync.dma_start(bhn, b_hn.to_broadcast([128, PD]))
        for t in range(2):
            for hi, (wh, bh, outdram) in enumerate(
                    ((whi, bhi, out_i), (whn, bhn, out_n))):
                o = tail.tile([128, PD], F32, tag="headout", name="headout")
                for n in range(2):
                    ps = ps_big.tile([128, 512], F32, tag="big", name="big")
                    for k in range(4):
                        nc.tensor.matmul(ps, zf[k][:, 128 * t:128 * (t + 1)],
                                         wh[:, k, 512 * n:512 * (n + 1)],
                                         start=(k == 0), stop=(k == 3))
                    nc.vector.tensor_add(o[:, 512 * n:512 * (n + 1)], ps,
                                         bh[:, 512 * n:512 * (n + 1)])
                nc.sync.dma_start(outdram[128 * t:128 * (t + 1), :], o)
        tail.release()

    nc.compile()
    return nc


# ---------------- host side ----------------

def _chunk_w(w, dtype=NBF):
    """[Din, Dout] -> [128, Din//128, Dout]"""
    din, dout = w.shape
    return np.ascontiguousarray(
        w.reshape(din // 128, 128, dout).transpose(1, 0, 2)).astype(dtype)


def _chunk_b(b, dtype=np.float32):
    """[Dout] -> [128, Dout//128] column-chunk layout"""
    return np.ascontiguousarray(b.reshape(-1, 128).T).astype(dtype)


def prep_inputs(inputs, n_enc=N_ENC, n_dec=N_DEC):
    f32 = np.float32
    g = {k: np.asarray(v, f32) if np.asarray(v).dtype != np.int32 else np.asarray(v)
         for k, v in inputs.items()}
    IMG, MAXD = 1024.0, 8.0
    coords, depths, mask = g["coords"], g["depths"], g["mask"]
    x1, x2, y1, y2 = coords[..., 0], coords[..., 1], coords[..., 2], coords[..., 3]
    posf = np.stack([(x1 + x2) / 2.0 / IMG, (y1 + y2) / 2.0 / IMG,
                     (x2 - x1) / IMG, (y2 - y1) / IMG, depths / MAXD], -1)
    patches = g["patches"].reshape(B, L, PD)
    visible = (mask == 0).astype(f32)  # [B, L]

    shared = {}
    shared["w_pe"] = _chunk_w(g["pe_w"])
    shared["b_embed"] = _chunk_b(g["pe_b"] + g["pos2_b"])
    shared["w_pos1"] = g["pos1_w"].astype(NBF)
    shared["b_pos1"] = _chunk_b(g["pos1_b"])
    shared["w_pos2"] = _chunk_w(g["pos2_w"])

    def layer_stack(n, lnw1, lnb1, qkvw, qkvb, prw, prb, lnw2, lnb2,
                    f1w, f1b, f2w, f2b, d_model):
        o = {k: [] for k in ("wqk", "bqk", "wv", "wpr", "bpr", "wf1", "bf1",
                             "wf2", "bf2")}
        for i in range(n):
            w_qk = lnw1[i][:, None] * qkvw[i][:, :2 * d_model]
            b_qk = lnb1[i] @ qkvw[i][:, :2 * d_model] + qkvb[i][:2 * d_model]
            w_v = lnw1[i][:, None] * qkvw[i][:, 2 * d_model:]
            b_v = lnb1[i] @ qkvw[i][:, 2 * d_model:] + qkvb[i][2 * d_model:]
            o["wqk"].append(_chunk_w(w_qk))
            o["bqk"].append(_chunk_b(b_qk))
            o["wv"].append(_chunk_w(w_v))
            o["wpr"].append(_chunk_w(prw[i]))
            o["bpr"].append(_chunk_b(prb[i] + b_v @ prw[i]))
            w_f1 = lnw2[i][:, None] * f1w[i]
            b_f1 = lnb2[i] @ f1w[i] + f1b[i]
            o["wf1"].append(_chunk_w(w_f1))
            o["bf1"].append(_chunk_b(b_f1))
            o["wf2"].append(_chunk_w(f2w[i]))
            o["bf2"].append(_chunk_b(f2b[i]))
        return {k: np.stack(v) for k, v in o.items()}

    enc = layer_stack(n_enc, g["e_ln1_w"], g["e_ln1_b"], g["e_qkv_w"], g["e_qkv_b"],
                      g["e_pr_w"], g["e_pr_b"], g["e_ln2_w"], g["e_ln2_b"],
                      g["e_f1_w"], g["e_f1_b"], g["e_f2_w"], g["e_f2_b"], D)
    dec = layer_stack(n_dec, g["d_ln1_w"], g["d_ln1_b"], g["d_qkv_w"], g["d_qkv_b"],
                      g["d_pr_w"], g["d_pr_b"], g["d_ln2_w"], g["d_ln2_b"],
                      g["d_f1_w"], g["d_f1_b"], g["d_f2_w"], g["d_f2_b"], DD)
    for k, v in enc.items():
        shared[f"e_{k}"] = v
    for k, v in dec.items():
        shared[f"d_{k}"] = v

    shared["enw"] = _chunk_b(g["enorm_w"])
    shared["enb"] = _chunk_b(g["enorm_b"])
    shared["w_de"] = _chunk_w(g["de_w"])
    shared["b_de"] = _chunk_b(g["de_b"])
    shared["w_dpos1"] = g["dpos1_w"].astype(NBF)
    shared["b_dpos1"] = _chunk_b(g["dpos1_b"])
    shared["w_dpos2"] = _chunk_w(g["dpos2_w"])
    shared["b_dpos2"] = _chunk_b(g["dpos2_b"])
    shared["w_hi"] = _chunk_w(g["dnorm_w"][:, None] * g["hi_w"])
    shared["b_hi"] = (g["dnorm_b"] @ g["hi_w"] + g["hi_b"]).astype(f32)[None, :]
    shared["w_hn"] = _chunk_w(g["dnorm_w"][:, None] * g["hn_w"])
    shared["b_hn"] = (g["dnorm_b"] @ g["hn_w"] + g["hn_b"]).astype(f32)[None, :]
    ind_e_np = np.zeros((2, 128), NBF)
    for j in range(2):
        ind_e_np[j, 64 * j:64 * (j + 1)] = 1
    shared["ind_e"] = ind_e_np
    ind_d_np = np.zeros((4, 128), NBF)
    for j in range(4):
        ind_d_np[j, 32 * j:32 * (j + 1)] = 1
    shared["ind_d"] = ind_d_np

    in_maps = []
    for c in range(8):
        b, h = c // 2, c % 2
        sl = slice(h * T, (h + 1) * T)
        m = dict(shared)
        m["patches_t"] = np.ascontiguousarray(
            patches[b, sl].T.reshape(8, 128, T).transpose(1, 0, 2)).astype(NBF)
        m["posf_t"] = np.ascontiguousarray(posf[b, sl].T).astype(NBF)
        vb = visible[b]
        m["mbias"] = np.ascontiguousarray(
            np.where(vb > 0.5, 0.0, MASK_BIAS).astype(f32).reshape(4, 128).T)
        vloc = visible[b, sl]
        m["vis"] = np.broadcast_to(vloc[None, :], (128, T)).astype(f32).copy()
        m["mtk"] = np.ascontiguousarray(
            (g["mask_token"].reshape(4, 128)[:, :, None] *
             (1.0 - vloc)[None, None, :]).transpose(1, 0, 2)).astype(f32)
        in_maps.append(m)
    return in_maps


_PROG = {}


def _get_prog(n_enc=N_ENC, n_dec=N_DEC):
    key = (n_enc, n_dec)
    if key not in _PROG:
        _PROG[key] = build_program(n_enc, n_dec)
    return _PROG[key]


def run(inputs, n_enc=N_ENC, n_dec=N_DEC, **kwargs):
    nc = _get_prog(n_enc, n_dec)
    in_maps = prep_inputs(inputs, n_enc, n_dec)
    res = run_bass_kernel_spmd(nc, in_maps, core_ids=list(range(8)), **kwargs)
    oi = np.zeros((B, L, PD), np.float32)
    on = np.zeros((B, L, PD), np.float32)
    for c in range(8):
        b, h = c // 2, c % 2
        oi[b, h * T:(h + 1) * T] = res.results[c]["out_i"]
        on[b, h * T:(h + 1) * T] = res.results[c]["out_n"]
    return (oi, on), res


def kernel(**inputs):
    (oi, on), _ = run(inputs)
    return oi, on



# revision 23
# speedup vs baseline: 1.1643x; 1.1643x over previous
"""MAE-ViT forward on 8 trn2 NeuronCores.

Sharding: data-parallel over B=4 samples x 2-way sequence split (256
tokens/core). Feature-major activations on-chip; bf16 matmuls with fp32
accumulation; fp32 residual stream. One K + one V AllGather (bf16)
between the two cores of each sample's pair per attention layer.
LayerNorm scales/biases folded into adjacent weights on the host.

Engine plan (v2): phase-batched attention (quadrant-packed score MMs,
[128,512] exp tiles, ones-in-V denominators, indicator-matrix broadcast
of 1/den), LayerNorm rstd via ln/exp on ScalarE (stays in the
natural_log_exp act table set with softmax exp), QK/f1 biases folded
into K=1 matmuls, PSUM drains on VectorE, weight DMA on the gpsimd
queue so the sync queue stays free for latency-critical transfers.
"""

import os
import numpy as np
import ml_dtypes

import concourse.bass as bass
import concourse.bacc as bacc
import concourse.tile as tile
import concourse.mybir as mybir
from concourse.bass_utils import run_bass_kernel_spmd

BF16 = mybir.dt.bfloat16
F32 = mybir.dt.float32
NBF = ml_dtypes.bfloat16
AF = mybir.ActivationFunctionType
ALU = mybir.AluOpType

# Model dims (hardcoded per problem spec)
B, L = 4, 512
T = 256           # tokens per core
D, DD = 768, 512
PD = 1024         # patch dim
N_ENC, N_DEC = 12, 8
ENC_H, DEC_H = 12, 16
ENC_HD, DEC_HD = 64, 32
EPS = 1e-6
MASK_BIAS = -80.0
GROUPS = [[0, 1], [2, 3], [4, 5], [6, 7]]
NO_CC = bool(os.environ.get("BISECT_NO_CC"))  # timeline-sim mode: replace AllGathers with local DMA copies


def _ln_to_z(nc, pools, x_tiles, F, z_dt=BF16, out_pool=None, out_tag="z"):
    """LayerNorm stats+apply in feature-major layout.

    x_tiles: F fp32 [128, T] tiles (features on partitions).
    Returns F z tiles of dtype z_dt with z = (x - mean) * rstd per token.
    rstd computed as exp(-0.5*ln(var+eps)) so ScalarE stays in the
    ln/exp table set.
    """
    sbuf, sq_pool = pools["sbuf"], pools["sq"]
    ones_col, ones_bf = pools["ones_col"], pools["ones_bf"]
    Dv = F * 128
    st = pools["ps_st"].tile([1, T], F32, tag="st", name="st")
    stq = pools["ps_bc"].tile([1, T], F32, tag="bc", name="stq")
    xb_tiles = []
    for k in range(F):
        xb = sq_pool.tile([128, T], BF16, tag="xb", name="xb")
        nc.vector.tensor_copy(xb, x_tiles[k])
        sq = sq_pool.tile([128, T], BF16, tag="sq", name="sq")
        nc.scalar.activation(sq, x_tiles[k], AF.Square)
        nc.tensor.matmul(st, ones_col[:, 0:1], xb,
                         start=(k == 0), stop=(k == F - 1))
        nc.tensor.matmul(stq, ones_col[:, 0:1], sq,
                         start=(k == 0), stop=(k == F - 1))
        xb_tiles.append(xb)
    mean = sbuf.tile([1, T], BF16, tag="ln_mean", name="ln_mean")
    nc.vector.tensor_scalar_mul(mean, st, 1.0 / Dv)
    m2 = sbuf.tile([1, T], F32, tag="ln_m2", name="ln_m2")
    nc.vector.tensor_mul(m2, mean, mean)
    var = sbuf.tile([1, T], F32, tag="ln_var", name="ln_var")
    nc.vector.scalar_tensor_tensor(var, stq, 1.0 / Dv, m2,
                                   ALU.mult, ALU.subtract)
    lnv = sbuf.tile([1, T], F32, tag="ln_lnv", name="ln_lnv")
    nc.scalar.activation(lnv, var, AF.Ln, bias=pools["eps"])
    rstd = sbuf.tile([1, T], BF16, tag="ln_rstd", name="ln_rstd")
    nc.scalar.activation(rstd, lnv, AF.Exp, scale=-0.5)
    # broadcast mean/rstd across partitions via K=1 bf16 outer products
    bc = pools["ps_bc"].tile([128, 2 * T], F32, tag="bc", name="bc")
    nc.tensor.matmul(bc[:, 0:T], ones_bf[0:1, :], mean, start=True, stop=True)
    nc.tensor.matmul(bc[:, T:2 * T], ones_bf[0:1, :], rstd, start=True, stop=True)
    z_tiles = []
    for k in range(F):
        t = sq_pool.tile([128, T], F32, tag="lnt", name="lnt")
        nc.vector.tensor_sub(t, x_tiles[k], bc[:, 0:T])
        zp = out_pool if out_pool is not None else pools["z"]
        z = zp.tile([128, T], z_dt, tag=f"{out_tag}{k}", name=f"{out_tag}{k}")
        nc.vector.tensor_mul(z, t, bc[:, T:2 * T])
        z_tiles.append(z)
    return z_tiles


def _attn_av(nc, a_tiles, v_full, av, h, hd, kp, KT):
    """Emit the AV matmuls for kt-pair kp of head h."""
    a_sb = a_tiles.pop(kp)
    for half in range(2):
        kt = 2 * kp + half
        nc.tensor.matmul(
            av[0:hd + 1, :],
            v_full[kt][:, h, :], a_sb[:, T * half:T * (half + 1)],
            start=(kt == 0), stop=(kt == KT - 1))


def _attention(nc, pools, z, F, nh, hd, wk, wqt, bqk, wv, vis_kv, cc, ind_sb):
    """Full attention for one layer. Returns attn output tiles (fm, bf16).

    Phase-batched: per head-group (one 128-row feature tile = hpt heads),
    quadrant-packed score MMs into a [128, hpt*T<=512] PSUM tile, one exp
    per (group, kt), AV with ones-in-V denominators, then a batched
    ln/exp reciprocal of all denominators and an indicator-matrix
    broadcast multiply.
    """
    sbuf = pools["sbuf"]
    ps_mm, ps_exp, ps_av = pools["ps_mm"], pools["ps_exp"], pools["ps_av"]
    ones_row = pools["ones_row"]
    Dm = F * 128
    KT = L // 128
    hpt = 128 // hd               # heads per 128-row tile (2 enc, 4 dec)
    scale = 1.0 / np.sqrt(hd)
    dram = cc["dram"]

    # --- K feature-major [Dm, T] -> cc-in; AllGather (issued early) ---
    k_cc_in = dram.tile([F, 128, T], BF16, tag="k_cc_in", name="k_cc_in")
    k_cc_out = dram.tile([2, F, 128, T], BF16, tag="k_cc_out", name="k_cc_out")
    for m in range(F):
        ps = ps_mm.tile([128, T], F32, tag="mm", name="mm")
        for k in range(F):
            nc.tensor.matmul(ps, wk[:, k, 128 * m:128 * (m + 1)],
                             z[k], start=(k == 0), stop=(k == F - 1))
        kl = pools["kq"].tile([128, T], BF16, tag=f"kl{m}", name=f"kl{m}")
        nc.scalar.activation(kl, ps, AF.Identity, bias=bqk[:, F + m:F + m + 1])
        nc.sync.dma_start(k_cc_in[m], kl)
    if NO_CC:
        nc.sync.dma_start(k_cc_out[0], k_cc_in[:])
        nc.sync.dma_start(k_cc_out[1], k_cc_in[:])
    else:
        nc.gpsimd.collective_compute(
            "AllGather", ALU.bypass, replica_groups=GROUPS,
            ins=[k_cc_in[:].opt()], outs=[k_cc_out[:].opt()])

    # --- V token-major [T, Dm] -> cc-in; AllGather ---
    v_cc_in = dram.tile([2, 128, Dm], BF16, tag="v_cc_in", name="v_cc_in")
    v_cc_out = dram.tile([2, 2, 128, Dm], BF16, tag="v_cc_out", name="v_cc_out")
    NV = min(Dm // 2, 512)
    ones_nh = pools["ones_nh"]
    for t in range(2):
        vl = pools["v"].tile([128, nh, hd + 1], BF16, tag=f"vl{t}", name=f"vl{t}")
        if vis_kv is None:
            nc.vector.memset(vl[:, :, hd:hd + 1], 1.0)
        else:
            nc.vector.tensor_scalar_mul(vl[:, :, hd:hd + 1],
                                        ones_nh[:, 0:nh, :],
                                        vis_kv[0][:, t:t + 1])
        hpn = NV // hd            # heads per NV block
        for n in range(Dm // NV):
            ps = ps_mm.tile([128, NV], F32, tag="mm", name="mm")
            for k in range(F):
                nc.tensor.matmul(ps, z[k][:, 128 * t:128 * (t + 1)],
                                 wv[:, k, NV * n:NV * (n + 1)],
                                 start=(k == 0), stop=(k == F - 1))
            if vis_kv is None:
                nc.vector.tensor_copy(
                    vl[:, hpn * n:hpn * (n + 1), 0:hd],
                    ps.rearrange("p (h d) -> p h d", h=hpn))
            else:
                nc.vector.tensor_scalar_mul(
                    vl[:, hpn * n:hpn * (n + 1), 0:hd],
                    ps.rearrange("p (h d) -> p h d", h=hpn),
                    vis_kv[0][:, t:t + 1])
        nc.sync.dma_start(
            v_cc_in[t].rearrange("p (h d) -> p h d", h=nh),
            vl[:, :, 0:hd])
    if NO_CC:
        nc.sync.dma_start(v_cc_out[0], v_cc_in[:])
        nc.sync.dma_start(v_cc_out[1], v_cc_in[:])
    else:
        nc.gpsimd.collective_compute(
            "AllGather", ALU.bypass, replica_groups=GROUPS,
            ins=[v_cc_in[:].opt()], outs=[v_cc_out[:].opt()])

    # --- Q feature-major (overlaps the collectives) ---
    q_sb = []
    for m in range(F):
        ps = ps_mm.tile([128, T], F32, tag="mm", name="mm")
        for k in range(F):
            nc.tensor.matmul(ps, wqt[:, k, 128 * m:128 * (m + 1)],
                             z[k], start=(k == 0), stop=(k == F - 1))
        q = pools["kq"].tile([128, T], BF16, tag=f"q{m}", name=f"q{m}")
        nc.scalar.activation(q, ps, AF.Identity, bias=bqk[:, m:m + 1])
        q_sb.append(q)

    # --- readback K_full [F][128, L] and V_full [KT][128, nh, hd+1] ---
    k_full = []
    for m in range(F):
        kf = pools["kf"].tile([128, L], BF16, tag=f"kf{m}", name=f"kf{m}")
        nc.sync.dma_start(kf[:, 0:T], k_cc_out[0, m])
        nc.sync.dma_start(kf[:, T:L], k_cc_out[1, m])
        k_full.append(kf)
    v_full = []
    for kt in range(KT):
        vf = pools["v"].tile([128, nh, hd + 1], BF16, tag=f"vf{kt}",
                             name=f"vf{kt}")
        if vis_kv is None:
            nc.vector.memset(vf[:, :, hd:hd + 1], 1.0)
        else:
            nc.vector.tensor_scalar_mul(vf[:, :, hd:hd + 1],
                                        ones_nh[:, 0:nh, :],
                                        vis_kv[1][:, kt:kt + 1])
        nc.sync.dma_start(
            vf[:, :, 0:hd],
            v_cc_out[kt // 2, kt % 2].rearrange("p (h d) -> p h d", h=nh))
        v_full.append(vf)

    if os.environ.get("BISECT_ATTN_QKV"):
        return q_sb
    # --- phase-batched scores / exp / AV ---
    # denominator grid [128, ceil(nh/4)*T]: head h at partition 32*(h%4),
    # columns (h//4)*T onward -- every engine AP stays 32-aligned.
    nhb = (nh + 3) // 4
    noden = bool(os.environ.get("BISECT_ATTN_NODEN"))
    den4 = pools["dn"].tile([128, nhb * T], F32, tag="den4", name="den4")
    if not noden:
        nc.vector.memset(den4, 1.0)
    uv_tiles = []
    for g in range(F):
        uv = pools["uv"].tile([128, T], BF16, tag=f"uv{g}", name=f"uv{g}")
        for j in range(hpt):
            h = g * hpt + j
            ro = j * hd
            # one av tile (= one PSUM bank) per head: single accumulation
            # group per bank; score kt-pairs share a bank but issue from
            # the same PE quadrant, so their drains are serialized
            av = ps_av.tile([128, T], F32, tag="av", name="av")
            a_tiles = {}
            for kp in range(KT // 2):
                s = ps_exp.tile([128, 2 * T], F32, tag="exp", name="exp")
                for half in range(2):
                    kt = 2 * kp + half
                    nc.tensor.matmul(
                        s[:, T * half:T * (half + 1)],
                        k_full[g][ro:ro + hd, 128 * kt:128 * (kt + 1)],
                        q_sb[g][ro:ro + hd, :], start=True, stop=True,
                        tile_position=(ro, 0))
                a_sb = pools["a"].tile([128, 2 * T], BF16, tag="a", name="a")
                nc.scalar.activation(a_sb, s, AF.Exp, scale=scale)
                a_tiles[kp] = a_sb
                if kp >= 1:
                    _attn_av(nc, a_tiles, v_full, av, h, hd, kp - 1, KT)
            _attn_av(nc, a_tiles, v_full, av, h, hd, KT // 2 - 1, KT)
            # drain AV: uv (attn rows, bf16) + ln(den) into the aligned grid
            nc.scalar.activation(uv[j * hd:(j + 1) * hd, :],
                                 av[0:hd, :], AF.Identity)
            if not noden:
                r0 = 32 * (h % 4)
                nc.scalar.activation(
                    den4[r0:r0 + 1, (h // 4) * T:(h // 4 + 1) * T],
                    av[hd:hd + 1, :], AF.Ln)
        uv_tiles.append(uv)

    if os.environ.get("BISECT_ATTN_NONORM"):
        return uv_tiles
    # batched 1/den = exp(-ln(den)) over the whole grid
    rcb = pools["dn"].tile([128, nhb * T], BF16, tag="rcb", name="rcb")
    nc.scalar.activation(rcb, den4, AF.Exp, scale=-1.0)

    # broadcast 1/den to head rows via aligned K=1 outer products; normalize
    ones_sq = pools["ones_sq"]
    attn = []
    for g in range(F):
        bca = pools["ps_bc"].tile([128, T], F32, tag="bc", name="bca")
        for j in range(hpt):
            h = g * hpt + j
            r0 = 32 * (h % 4)
            nc.tensor.matmul(bca[j * hd:(j + 1) * hd, :],
                             ones_sq[r0:r0 + 1, 0:hd],
                             rcb[r0:r0 + 1, (h // 4) * T:(h // 4 + 1) * T],
                             start=True, stop=True,
                             tile_position=(r0, j * hd))
        a = sbuf.tile([128, T], BF16, tag=f"attn{g}", name=f"attn{g}")
        nc.vector.tensor_mul(a, uv_tiles[g], bca)
        attn.append(a)
    return attn


def _layer(nc, pools, cc, x_tiles, F, n_heads, hd, waps, vis_kv, ind_sb,
           wpools):
    """One transformer block (attn + MLP) updating x_tiles in place."""
    wq_pool, wv_pool, wp_pool, w1_pool, w2_pool, bias_pool = wpools
    (a_wqk, a_bqk, a_wv, a_wpr, a_bpr, a_wf1, a_bf1w, a_wf2, a_bf2) = waps
    ps_mm = pools["ps_mm"]
    ones_row = pools["ones_row"]
    Dm = F * 128
    F1 = a_wf1.shape[-1] // 128   # hidden tiles (24 enc / 16 dec)

    wk = wq_pool.tile([128, F, Dm], BF16, tag="wk", name="wk")
    nc.sync.dma_start(wk, a_wqk[:, :, Dm:2 * Dm])
    wqt = wv_pool.tile([128, F, Dm], BF16, tag="wqt", name="wqt")
    nc.sync.dma_start(wqt, a_wqk[:, :, 0:Dm])
    wv = wv_pool.tile([128, F, Dm], BF16, tag="wv", name="wv")
    nc.sync.dma_start(wv, a_wv)
    wpr = wp_pool.tile([128, F, Dm], BF16, tag="wpr", name="wpr")
    nc.sync.dma_start(wpr, a_wpr)
    bqk = bias_pool.tile([128, 2 * F], F32, tag="bqk", name="bqk")
    nc.sync.dma_start(bqk, a_bqk)
    bf1w = bias_pool.tile([1, F1 * 128], BF16, tag="bf1w", name="bf1w")
    nc.sync.dma_start(bf1w, a_bf1w)
    bpr = bias_pool.tile([128, F], F32, tag="bpr", name="bpr")
    nc.sync.dma_start(bpr, a_bpr)
    bf2 = bias_pool.tile([128, F], F32, tag="bf2", name="bf2")
    nc.sync.dma_start(bf2, a_bf2)

    z = _ln_to_z(nc, pools, x_tiles, F)
    if os.environ.get("BISECT_SKIP_ATTN"):
        attn = z
    else:
        attn = _attention(nc, pools, z, F, n_heads, hd, wk, wqt, bqk, wv,
                          vis_kv, cc, ind_sb)
    for m in range(F):
        ps = ps_mm.tile([128, T], F32, tag="mm", name="mm")
        for k in range(F):
            nc.tensor.matmul(ps, wpr[:, k, 128 * m:128 * (m + 1)],
                             attn[k], start=(k == 0), stop=(k == F - 1))
        nc.vector.scalar_tensor_tensor(x_tiles[m], ps, bpr[:, m:m + 1],
                                       x_tiles[m], ALU.add, ALU.add)
    if os.environ.get("BISECT_SKIP_MLP"):
        return
    z2 = _ln_to_z(nc, pools, x_tiles, F)

    # f1 in pairs of m-tiles sharing a [128, 512] PSUM bank; bias folded
    # as a K=1 matmul so one GELU covers both halves
    hmid = []
    mpb = F1 // 4
    for b in range(4):
        w1b = w1_pool.tile([128, F, 128 * mpb], BF16, tag="wf1", name="wf1")
        nc.sync.dma_start(w1b, a_wf1[:, :, 128 * mpb * b:128 * mpb * (b + 1)])
        for mp in range(mpb // 2):
            ps = ps_mm.tile([128, 2 * T], F32, tag="mm", name="mm")
            for half in range(2):
                m = 2 * mp + half
                mg = b * mpb + m
                for k in range(F):
                    nc.tensor.matmul(ps[:, T * half:T * (half + 1)],
                                     w1b[:, k, 128 * m:128 * (m + 1)],
                                     z2[k], start=(k == 0), stop=False)
                nc.tensor.matmul(ps[:, T * half:T * (half + 1)],
                                 bf1w[0:1, 128 * mg:128 * (mg + 1)],
                                 ones_row[0:1, :], start=False, stop=True)
            hm = pools["h"].tile([128, 2 * T], BF16, tag=f"hm{b * mpb // 2 + mp}",
                                 name=f"hm{mg}")
            nc.scalar.activation(hm, ps, AF.Gelu)
            hmid.append(hm)
    # f2 streamed in column-blocks of 2 m-tiles
    for b in range(F // 2):
        w2b = w2_pool.tile([128, F1, 256], BF16, tag="wf2", name="wf2")
        nc.sync.dma_start(w2b, a_wf2[:, :, 256 * b:256 * (b + 1)])
        for mm in range(2):
            m = 2 * b + mm
            ps = ps_mm.tile([128, T], F32, tag="mm", name="mm")
            for k in range(F1 // 2):
                for half in range(2):
                    nc.tensor.matmul(ps, w2b[:, 2 * k + half, 128 * mm:128 * (mm + 1)],
                                     hmid[k][:, T * half:T * (half + 1)],
                                     start=(k == 0 and half == 0),
                                     stop=(k == F1 // 2 - 1 and half == 1))
            nc.vector.scalar_tensor_tensor(x_tiles[m], ps, bf2[:, m:m + 1],
                                           x_tiles[m], ALU.add, ALU.add)


def build_program(n_enc=N_ENC, n_dec=N_DEC, no_cc=False):
    global NO_CC
    NO_CC = no_cc
    nc = bacc.Bacc("TRN2", target_bir_lowering=False, debug=False, num_devices=8)

    def inp(name, shape, dt=BF16):
        return nc.dram_tensor(name, shape, dt, kind="ExternalInput").ap()

    # --- inputs (per-core) ---
    patches_t = inp("patches_t", [128, 8, T])
    posf_t = inp("posf_t", [5, T])
    w_pe = inp("w_pe", [128, 8, D])
    b_embed = inp("b_embed", [128, 6], F32)
    w_pos1 = inp("w_pos1", [5, 384])
    b_pos1 = inp("b_pos1", [128, 3], F32)
    w_pos2 = inp("w_pos2", [128, 3, D])
    if n_enc:
        e_wqk = inp("e_wqk", [n_enc, 128, 6, 1536])
        e_bqk = inp("e_bqk", [n_enc, 128, 12], F32)
        e_wv = inp("e_wv", [n_enc, 128, 6, D])
        e_wpr = inp("e_wpr", [n_enc, 128, 6, D])
        e_bpr = inp("e_bpr", [n_enc, 128, 6], F32)
        e_wf1 = inp("e_wf1", [n_enc, 128, 6, 3072])
        e_bf1w = inp("e_bf1w", [n_enc, 1, 3072])
        e_wf2 = inp("e_wf2", [n_enc, 128, 24, D])
        e_bf2 = inp("e_bf2", [n_enc, 128, 6], F32)
    vis_loc = inp("vis_loc", [128, 2], F32)
    vis_glob = inp("vis_glob", [128, 4], F32)
    vis = inp("vis", [128, T], F32)
    enw = inp("enw", [128, 6], F32)
    enb = inp("enb", [128, 6], F32)
    w_de = inp("w_de", [128, 6, DD])
    b_de = inp("b_de", [128, 4], F32)
    mtk = inp("mtk", [128, 4, T])
    w_dpos1 = inp("w_dpos1", [5, 256])
    b_dpos1 = inp("b_dpos1", [128, 2], F32)
    w_dpos2 = inp("w_dpos2", [128, 2, DD])
    b_dpos2 = inp("b_dpos2", [128, 4], F32)
    if n_dec:
        d_wqk = inp("d_wqk", [n_dec, 128, 4, 1024])
        d_bqk = inp("d_bqk", [n_dec, 128, 8], F32)
        d_wv = inp("d_wv", [n_dec, 128, 4, DD])
        d_wpr = inp("d_wpr", [n_dec, 128, 4, DD])
        d_bpr = inp("d_bpr", [n_dec, 128, 4], F32)
        d_wf1 = inp("d_wf1", [n_dec, 128, 4, 2048])
        d_bf1w = inp("d_bf1w", [n_dec, 1, 2048])
        d_wf2 = inp("d_wf2", [n_dec, 128, 16, DD])
        d_bf2 = inp("d_bf2", [n_dec, 128, 4], F32)
    w_hi = inp("w_hi", [128, 4, PD])
    w_hn = inp("w_hn", [128, 4, PD])
    b_hi = inp("b_hi", [1, PD], F32)
    b_hn = inp("b_hn", [1, PD], F32)
    ind_e_in = inp("ind_e", [2, 128])
    ind_d_in = inp("ind_d", [4, 128])
    out_i = nc.dram_tensor("out_i", [T, PD], F32, kind="ExternalOutput").ap()
    out_n = nc.dram_tensor("out_n", [T, PD], F32, kind="ExternalOutput").ap()

    from contextlib import ExitStack
    with tile.TileContext(nc) as tc, ExitStack() as es:
        sbuf = es.enter_context(tc.tile_pool(name="sbuf", bufs=1))
        consts = es.enter_context(tc.tile_pool(name="consts", bufs=1))
        xpool = es.enter_context(tc.tile_pool(name="x", bufs=1))
        zpool = es.enter_context(tc.tile_pool(name="z", bufs=2))
        sqpool = es.enter_context(tc.tile_pool(name="sq", bufs=3))
        kqpool = es.enter_context(tc.tile_pool(name="kq", bufs=1))
        kfpool = es.enter_context(tc.tile_pool(name="kf", bufs=1))
        vpool = es.enter_context(tc.tile_pool(name="v", bufs=1))
        apool = es.enter_context(tc.tile_pool(name="a", bufs=4))
        uvpool = es.enter_context(tc.tile_pool(name="uv", bufs=1))
        dnpool = es.enter_context(tc.tile_pool(name="dn", bufs=1))
        hpool = es.enter_context(tc.tile_pool(name="h", bufs=1))
        wq_pool = es.enter_context(tc.tile_pool(name="wq", bufs=2))
        wv_pool = es.enter_context(tc.tile_pool(name="wv", bufs=1))
        wp_pool = es.enter_context(tc.tile_pool(name="wp", bufs=1))
        w1_pool = es.enter_context(tc.tile_pool(name="w1", bufs=2))
        w2_pool = es.enter_context(tc.tile_pool(name="w2", bufs=2))
        bias_pool = es.enter_context(tc.tile_pool(name="bias", bufs=1))
        ps_mm = es.enter_context(tc.tile_pool(name="ps_mm", bufs=2, space="PSUM"))
        ps_exp = es.enter_context(tc.tile_pool(name="ps_exp", bufs=2, space="PSUM"))
        ps_av = es.enter_context(tc.tile_pool(name="ps_av", bufs=2, space="PSUM"))
        ps_st = es.enter_context(tc.tile_pool(name="ps_st", bufs=1, space="PSUM"))
        ps_bc = es.enter_context(tc.tile_pool(name="ps_bc", bufs=1, space="PSUM"))
        dram = es.enter_context(tc.tile_pool(name="dram", bufs=2, space="DRAM"))

        pools = dict(sbuf=sbuf, z=zpool, sq=sqpool, kq=kqpool, kf=kfpool,
                     v=vpool, a=apool, uv=uvpool, dn=dnpool, h=hpool,
                     ps_mm=ps_mm, ps_exp=ps_exp, ps_av=ps_av, ps_st=ps_st,
                     ps_bc=ps_bc)
        cc = dict(dram=dram)
        wpools = (wq_pool, wv_pool, wp_pool, w1_pool, w2_pool, bias_pool)

        ones_col = consts.tile([128, 1], BF16)
        nc.vector.memset(ones_col, 1.0)
        ones_row = consts.tile([1, T], BF16)
        nc.vector.memset(ones_row, 1.0)
        ones_bf = consts.tile([1, 128], BF16)
        nc.vector.memset(ones_bf, 1.0)
        eps_sb = consts.tile([1, 1], F32)
        nc.vector.memset(eps_sb, EPS)
        pools["eps"] = eps_sb
        pools["ones_col"] = ones_col
        pools["ones_row"] = ones_row
        pools["ones_bf"] = ones_bf

        vis_loc_sb = consts.tile([128, 2], F32)
        nc.sync.dma_start(vis_loc_sb, vis_loc)
        vis_glob_sb = consts.tile([128, 4], F32)
        nc.sync.dma_start(vis_glob_sb, vis_glob)
        ones_nh = consts.tile([128, 16, 1], BF16)
        nc.vector.memset(ones_nh, 1.0)
        pools["ones_nh"] = ones_nh
        vis_kv = (vis_loc_sb, vis_glob_sb)
        vis_sb = consts.tile([128, T], F32)
        nc.sync.dma_start(vis_sb, vis)
        pf = consts.tile([5, T], BF16)
        nc.sync.dma_start(pf, posf_t)
        ones_sq = consts.tile([128, 128], BF16)
        nc.vector.memset(ones_sq, 1.0)
        pools["ones_sq"] = ones_sq

        # ===== embedding (scoped pool, released after) =====
        embed_pool = tc.alloc_tile_pool(name="embed", bufs=1)
        wp1 = embed_pool.tile([5, 384], BF16)
        nc.sync.dma_start(wp1, w_pos1)
        wp2 = embed_pool.tile([128, 3, D], BF16)
        nc.sync.dma_start(wp2, w_pos2)
        bp1 = embed_pool.tile([128, 3], F32)
        nc.sync.dma_start(bp1, b_pos1)
        pt = embed_pool.tile([128, 8, T], BF16)
        nc.sync.dma_start(pt, patches_t)
        wpe = embed_pool.tile([128, 8, D], BF16)
        nc.sync.dma_start(wpe, w_pe)
        bemb = embed_pool.tile([128, 6], F32)
        nc.sync.dma_start(bemb, b_embed)

        h1 = []
        for m in range(3):
            ps = ps_mm.tile([128, T], F32, tag="mm", name="mm")
            nc.tensor.matmul(ps, wp1[:, 128 * m:128 * (m + 1)], pf,
                             start=True, stop=True)
            t = embed_pool.tile([128, T], BF16, tag=f"h1_{m}", name=f"h1_{m}")
            nc.scalar.activation(t, ps, AF.Gelu, bias=bp1[:, m:m + 1])
            h1.append(t)

        x_tiles = [xpool.tile([128, T], F32, tag=f"x{k}", name=f"x{k}")
                   for k in range(6)]
        for m in range(6):
            ps = ps_mm.tile([128, T], F32, tag="mm", name="mm")
            for k in range(8):
                nc.tensor.matmul(ps, wpe[:, k, 128 * m:128 * (m + 1)],
                                 pt[:, k, :], start=(k == 0), stop=False)
            for k in range(3):
                nc.tensor.matmul(ps, wp2[:, k, 128 * m:128 * (m + 1)],
                                 h1[k], start=False, stop=(k == 2))
            nc.scalar.activation(x_tiles[m], ps, AF.Identity, bias=bemb[:, m:m + 1])
        embed_pool.release()

        # ===== encoder =====
        for i in range(n_enc):
            waps = (e_wqk[i], e_bqk[i], e_wv[i], e_wpr[i], e_bpr[i],
                    e_wf1[i], e_bf1w[i], e_wf2[i], e_bf2[i])
            _layer(nc, pools, cc, x_tiles, 6, ENC_H, ENC_HD, waps,
                   vis_kv, None, wpools)

        # ===== bridge: enc norm + mask + decoder embed (scoped pool) =====
        bridge = tc.alloc_tile_pool(name="bridge", bufs=1)
        enw_sb = bridge.tile([128, 6], F32)
        nc.sync.dma_start(enw_sb, enw)
        enb_sb = bridge.tile([128, 6], F32)
        nc.sync.dma_start(enb_sb, enb)
        ze = _ln_to_z(nc, pools, x_tiles, 6, out_pool=bridge, out_tag="ze")
        enc_sb = []
        for k in range(6):
            t = bridge.tile([128, T], F32, tag=f"enc_t{k}", name=f"enc_t{k}")
            nc.scalar.activation(t, ze[k], AF.Identity, bias=enb_sb[:, k:k + 1],
                                 scale=enw_sb[:, k:k + 1])
            e = bridge.tile([128, T], BF16, tag=f"enc{k}", name=f"enc{k}")
            nc.vector.tensor_mul(e, t, vis_sb)
            enc_sb.append(e)

        wde = bridge.tile([128, 6, DD], BF16)
        nc.sync.dma_start(wde, w_de)
        bde = bridge.tile([128, 4], F32)
        nc.sync.dma_start(bde, b_de)
        mtk_sb = bridge.tile([128, 4, T], BF16)
        nc.sync.dma_start(mtk_sb, mtk)
        wd1 = bridge.tile([5, 256], BF16)
        nc.sync.dma_start(wd1, w_dpos1)
        bd1 = bridge.tile([128, 2], F32)
        nc.sync.dma_start(bd1, b_dpos1)
        wd2 = bridge.tile([128, 2, DD], BF16)
        nc.sync.dma_start(wd2, w_dpos2)
        bd2 = bridge.tile([128, 4], F32)
        nc.sync.dma_start(bd2, b_dpos2)

        h1d = []
        for m in range(2):
            ps = ps_mm.tile([128, T], F32, tag="mm", name="mm")
            nc.tensor.matmul(ps, wd1[:, 128 * m:128 * (m + 1)], pf,
                             start=True, stop=True)
            t = bridge.tile([128, T], BF16, tag=f"h1d_{m}", name=f"h1d_{m}")
            nc.scalar.activation(t, ps, AF.Gelu, bias=bd1[:, m:m + 1])
            h1d.append(t)

        xd_tiles = [xpool.tile([128, T], F32, tag=f"xd{k}", name=f"xd{k}")
                    for k in range(4)]
        for m in range(4):
            ps = ps_mm.tile([128, T], F32, tag="mm", name="mm")
            for k in range(6):
                nc.tensor.matmul(ps, wde[:, k, 128 * m:128 * (m + 1)],
                                 enc_sb[k], start=(k == 0), stop=(k == 5))
            t1 = bridge.tile([128, T], F32, tag="dec_t1", name="dec_t1")
            nc.scalar.activation(t1, ps, AF.Identity, bias=bde[:, m:m + 1])
            nc.vector.tensor_mul(t1, t1, vis_sb)
            nc.vector.tensor_add(t1, t1, mtk_sb[:, m, :])
            ps2 = ps_mm.tile([128, T], F32, tag="mm", name="mm")
            for k in range(2):
                nc.tensor.matmul(ps2, wd2[:, k, 128 * m:128 * (m + 1)],
                                 h1d[k], start=(k == 0), stop=(k == 1))
            t2 = bridge.tile([128, T], F32, tag="dec_t2", name="dec_t2")
            nc.scalar.activation(t2, ps2, AF.Identity, bias=bd2[:, m:m + 1])
            nc.vector.tensor_add(xd_tiles[m], t1, t2)
        bridge.release()

        # ===== decoder =====
        for i in range(n_dec):
            waps = (d_wqk[i], d_bqk[i], d_wv[i], d_wpr[i], d_bpr[i],
                    d_wf1[i], d_bf1w[i], d_wf2[i], d_bf2[i])
            _layer(nc, pools, cc, xd_tiles, 4, DEC_H, DEC_HD, waps,
                   None, None, wpools)

        # ===== final norm + heads (token-major output) =====
        tail = tc.alloc_tile_pool(name="tail", bufs=2)
        zf = _ln_to_z(nc, pools, xd_tiles, 4)
        for (a_wh, a_bh, outdram) in ((w_hi, b_hi, out_i), (w_hn, b_hn, out_n)):
            wh = tail.tile([128, 4, PD], BF16, tag="wh", name="wh")
            nc.sync.dma_start(wh, a_wh)
            bh = tail.tile([128, PD], F32, tag="bh", name="bh")
            nc.sync.dma_start(bh, a_bh.to_broadcast([128, PD]))
            for t in range(2):
                for n in range(2):
                    ps = ps_exp.tile([128, 512], F32, tag="exp", name="head_ps")
                    for k in range(4):
                        nc.tensor.matmul(ps, zf[k][:, 128 * t:128 * (t + 1)],
                                         wh[:, k, 512 * n:512 * (n + 1)],
                                         start=(k == 0), stop=(k == 3))
                    o = tail.tile([128, 512], F32, tag="headout", name="headout")
                    nc.vector.tensor_add(o, ps, bh[:, 512 * n:512 * (n + 1)])
                    nc.sync.dma_start(
                        outdram[128 * t:128 * (t + 1), 512 * n:512 * (n + 1)], o)
        tail.release()

    nc.compile()
    return nc


# ---------------- host side ----------------

def _chunk_w(w, dtype=NBF):
    """[Din, Dout] -> [128, Din//128, Dout]"""
    din, dout = w.shape
    return np.ascontiguousarray(
        w.reshape(din // 128, 128, dout).transpose(1, 0, 2)).astype(dtype)


def _chunk_b(b, dtype=np.float32):
    """[Dout] -> [128, Dout//128] column-chunk layout"""
    return np.ascontiguousarray(b.reshape(-1, 128).T).astype(dtype)


def prep_inputs(inputs, n_enc=N_ENC, n_dec=N_DEC):
    f32 = np.float32
    g = {k: np.asarray(v, f32) if np.asarray(v).dtype != np.int32 else np.asarray(v)
         for k, v in inputs.items()}
    IMG, MAXD = 1024.0, 8.0
    coords, depths, mask = g["coords"], g["depths"], g["mask"]
    x1, x2, y1, y2 = coords[..., 0], coords[..., 1], coords[..., 2], coords[..., 3]
    posf = np.stack([(x1 + x2) / 2.0 / IMG, (y1 + y2) / 2.0 / IMG,
                     (x2 - x1) / IMG, (y2 - y1) / IMG, depths / MAXD], -1)
    patches = g["patches"].reshape(B, L, PD)
    visible = (mask == 0).astype(f32)  # [B, L]

    shared = {}
    shared["w_pe"] = _chunk_w(g["pe_w"])
    shared["b_embed"] = _chunk_b(g["pe_b"] + g["pos2_b"])
    shared["w_pos1"] = g["pos1_w"].astype(NBF)
    shared["b_pos1"] = _chunk_b(g["pos1_b"])
    shared["w_pos2"] = _chunk_w(g["pos2_w"])

    def layer_stack(n, lnw1, lnb1, qkvw, qkvb, prw, prb, lnw2, lnb2,
                    f1w, f1b, f2w, f2b, d_model):
        o = {k: [] for k in ("wqk", "bqk", "wv", "wpr", "bpr", "wf1", "bf1w",
                             "wf2", "bf2")}
        if n == 0:
            return {}
        for i in range(n):
            w_qk = lnw1[i][:, None] * qkvw[i][:, :2 * d_model]
            b_qk = lnb1[i] @ qkvw[i][:, :2 * d_model] + qkvb[i][:2 * d_model]
            w_v = lnw1[i][:, None] * qkvw[i][:, 2 * d_model:]
            b_v = lnb1[i] @ qkvw[i][:, 2 * d_model:] + qkvb[i][2 * d_model:]
            o["wqk"].append(_chunk_w(w_qk))
            o["bqk"].append(_chunk_b(b_qk))
            o["wv"].append(_chunk_w(w_v))
            o["wpr"].append(_chunk_w(prw[i]))
            o["bpr"].append(_chunk_b(prb[i] + b_v @ prw[i]))
            w_f1 = lnw2[i][:, None] * f1w[i]
            b_f1 = lnb2[i] @ f1w[i] + f1b[i]
            o["wf1"].append(_chunk_w(w_f1))
            o["bf1w"].append(b_f1.astype(NBF)[None, :])
            o["wf2"].append(_chunk_w(f2w[i]))
            o["bf2"].append(_chunk_b(f2b[i]))
        return {k: np.stack(v) for k, v in o.items()}

    enc = layer_stack(n_enc, g["e_ln1_w"], g["e_ln1_b"], g["e_qkv_w"], g["e_qkv_b"],
                      g["e_pr_w"], g["e_pr_b"], g["e_ln2_w"], g["e_ln2_b"],
                      g["e_f1_w"], g["e_f1_b"], g["e_f2_w"], g["e_f2_b"], D)
    dec = layer_stack(n_dec, g["d_ln1_w"], g["d_ln1_b"], g["d_qkv_w"], g["d_qkv_b"],
                      g["d_pr_w"], g["d_pr_b"], g["d_ln2_w"], g["d_ln2_b"],
                      g["d_f1_w"], g["d_f1_b"], g["d_f2_w"], g["d_f2_b"], DD)
    for k, v in enc.items():
        shared[f"e_{k}"] = v
    for k, v in dec.items():
        shared[f"d_{k}"] = v

    shared["enw"] = _chunk_b(g["enorm_w"])
    shared["enb"] = _chunk_b(g["enorm_b"])
    shared["w_de"] = _chunk_w(g["de_w"])
    shared["b_de"] = _chunk_b(g["de_b"])
    shared["w_dpos1"] = g["dpos1_w"].astype(NBF)
    shared["b_dpos1"] = _chunk_b(g["dpos1_b"])
    shared["w_dpos2"] = _chunk_w(g["dpos2_w"])
    shared["b_dpos2"] = _chunk_b(g["dpos2_b"])
    shared["w_hi"] = _chunk_w(g["dnorm_w"][:, None] * g["hi_w"])
    shared["b_hi"] = (g["dnorm_b"] @ g["hi_w"] + g["hi_b"]).astype(f32)[None, :]
    shared["w_hn"] = _chunk_w(g["dnorm_w"][:, None] * g["hn_w"])
    shared["b_hn"] = (g["dnorm_b"] @ g["hn_w"] + g["hn_b"]).astype(f32)[None, :]
    ind_e_np = np.zeros((2, 128), NBF)
    for j in range(2):
        ind_e_np[j, 64 * j:64 * (j + 1)] = 1
    shared["ind_e"] = ind_e_np
    ind_d_np = np.zeros((4, 128), NBF)
    for j in range(4):
        ind_d_np[j, 32 * j:32 * (j + 1)] = 1
    shared["ind_d"] = ind_d_np

    in_maps = []
    for c in range(8):
        b, h = c // 2, c % 2
        sl = slice(h * T, (h + 1) * T)
        m = dict(shared)
        m["patches_t"] = np.ascontiguousarray(
            patches[b, sl].T.reshape(8, 128, T).transpose(1, 0, 2)).astype(NBF)
        m["posf_t"] = np.ascontiguousarray(posf[b, sl].T).astype(NBF)
        vb = visible[b]
        m["vis_glob"] = np.ascontiguousarray(vb.astype(f32).reshape(4, 128).T)
        m["vis_loc"] = np.ascontiguousarray(
            visible[b, sl].astype(f32).reshape(2, 128).T)
        vloc = visible[b, sl]
        m["vis"] = np.broadcast_to(vloc[None, :], (128, T)).astype(f32).copy()
        m["mtk"] = np.ascontiguousarray(
            (g["mask_token"].reshape(4, 128)[:, :, None] *
             (1.0 - vloc)[None, None, :]).transpose(1, 0, 2)).astype(NBF)
        in_maps.append(m)
    return in_maps


_PROG = {}


def _get_prog(n_enc=N_ENC, n_dec=N_DEC):
    key = (n_enc, n_dec)
    if key not in _PROG:
        _PROG[key] = build_program(n_enc, n_dec)
    return _PROG[key]


def run(inputs, n_enc=N_ENC, n_dec=N_DEC, **kwargs):
    nc = _get_prog(n_enc, n_dec)
    in_maps = prep_inputs(inputs, n_enc, n_dec)
    res = run_bass_kernel_spmd(nc, in_maps, core_ids=list(range(8)), **kwargs)
    oi = np.zeros((B, L, PD), np.float32)
    on = np.zeros((B, L, PD), np.float32)
    for c in range(8):
        b, h = c // 2, c % 2
        oi[b, h * T:(h + 1) * T] = res.results[c]["out_i"]
        on[b, h * T:(h + 1) * T] = res.results[c]["out_n"]
    return (oi, on), res


def kernel(**inputs):
    (oi, on), _ = run(inputs)
    return oi, on


# revision 24
# speedup vs baseline: 1.4067x; 1.2082x over previous
"""MAE-ViT forward on 8 trn2 NeuronCores.

Sharding: data-parallel over B=4 samples x 2-way sequence split (256
tokens/core). Feature-major activations on-chip; bf16 matmuls with fp32
accumulation; fp32 residual stream. One K + one V AllGather (bf16)
between the two cores of each sample's pair per attention layer.
LayerNorm scales/biases folded into adjacent weights on the host.

Engine plan (v2): phase-batched attention (quadrant-packed score MMs,
[128,512] exp tiles, ones-in-V denominators, indicator-matrix broadcast
of 1/den), LayerNorm rstd via ln/exp on ScalarE (stays in the
natural_log_exp act table set with softmax exp), QK/f1 biases folded
into K=1 matmuls, PSUM drains on VectorE, weight DMA on the gpsimd
queue so the sync queue stays free for latency-critical transfers.
"""

import os
import numpy as np
import ml_dtypes

import concourse.bass as bass
import concourse.bacc as bacc
import concourse.tile as tile
import concourse.mybir as mybir
from concourse.bass_utils import run_bass_kernel_spmd

BF16 = mybir.dt.bfloat16
F32 = mybir.dt.float32
NBF = ml_dtypes.bfloat16
AF = mybir.ActivationFunctionType
ALU = mybir.AluOpType

# Model dims (hardcoded per problem spec)
B, L = 4, 512
T = 256           # tokens per core
D, DD = 768, 512
PD = 1024         # patch dim
N_ENC, N_DEC = 12, 8
ENC_H, DEC_H = 12, 16
ENC_HD, DEC_HD = 64, 32
EPS = 1e-6
MASK_BIAS = -80.0
GROUPS = [[0, 1], [2, 3], [4, 5], [6, 7]]
NO_CC = bool(os.environ.get("BISECT_NO_CC"))  # timeline-sim mode: replace AllGathers with local DMA copies


def _ln_to_z(nc, pools, x_tiles, F, z_dt=BF16, out_pool=None, out_tag="z"):
    """LayerNorm stats+apply in feature-major layout.

    x_tiles: F fp32 [128, T] tiles (features on partitions).
    Returns F z tiles of dtype z_dt with z = (x - mean) * rstd per token.
    rstd computed as exp(-0.5*ln(var+eps)) so ScalarE stays in the
    ln/exp table set.
    """
    sbuf, sq_pool = pools["sbuf"], pools["sq"]
    ones_col, ones_bf = pools["ones_col"], pools["ones_bf"]
    Dv = F * 128
    st = pools["ps_st"].tile([1, T], F32, tag="st", name="st")
    stq = pools["ps_bc"].tile([1, T], F32, tag="bc", name="stq")
    xb_tiles = []
    for k in range(F):
        xb = sq_pool.tile([128, T], BF16, tag="xb", name="xb")
        nc.vector.tensor_copy(xb, x_tiles[k])
        sq = sq_pool.tile([128, T], BF16, tag="sq", name="sq")
        nc.scalar.activation(sq, x_tiles[k], AF.Square)
        nc.tensor.matmul(st, ones_col[:, 0:1], xb,
                         start=(k == 0), stop=(k == F - 1))
        nc.tensor.matmul(stq, ones_col[:, 0:1], sq,
                         start=(k == 0), stop=(k == F - 1))
        xb_tiles.append(xb)
    mean = sbuf.tile([1, T], BF16, tag="ln_mean", name="ln_mean")
    nc.vector.tensor_scalar_mul(mean, st, 1.0 / Dv)
    m2 = sbuf.tile([1, T], F32, tag="ln_m2", name="ln_m2")
    nc.vector.tensor_mul(m2, mean, mean)
    var = sbuf.tile([1, T], F32, tag="ln_var", name="ln_var")
    nc.vector.scalar_tensor_tensor(var, stq, 1.0 / Dv, m2,
                                   ALU.mult, ALU.subtract)
    sd = sbuf.tile([1, T], F32, tag="ln_sd", name="ln_sd")
    nc.scalar.activation(sd, var, AF.Sqrt, bias=pools["eps"])
    rstdf = sbuf.tile([1, T], F32, tag="ln_rstdf", name="ln_rstdf")
    nc.vector.reciprocal_approx_fast(rstdf, sd)
    rstd = sbuf.tile([1, T], BF16, tag="ln_rstd", name="ln_rstd")
    nc.vector.tensor_copy(rstd, rstdf)
    # broadcast mean/rstd across partitions via K=1 bf16 outer products
    bc = pools["ps_bc"].tile([128, 2 * T], F32, tag="bc", name="bc")
    nc.tensor.matmul(bc[:, 0:T], ones_bf[0:1, :], mean, start=True, stop=True)
    nc.tensor.matmul(bc[:, T:2 * T], ones_bf[0:1, :], rstd, start=True, stop=True)
    z_tiles = []
    for k in range(F):
        t = sq_pool.tile([128, T], F32, tag="lnt", name="lnt")
        nc.vector.tensor_sub(t, x_tiles[k], bc[:, 0:T])
        zp = out_pool if out_pool is not None else pools["z"]
        z = zp.tile([128, T], z_dt, tag=f"{out_tag}{k}", name=f"{out_tag}{k}")
        nc.vector.tensor_mul(z, t, bc[:, T:2 * T])
        z_tiles.append(z)
    return z_tiles


def _attn_av(nc, a_tiles, v_full, av, h, hd, kp, KT):
    """Emit the AV matmuls for kt-pair kp of head h."""
    a_sb = a_tiles.pop(kp)
    for half in range(2):
        kt = 2 * kp + half
        nc.tensor.matmul(
            av[0:hd + 1, :],
            v_full[kt][:, h, :], a_sb[:, T * half:T * (half + 1)],
            start=(kt == 0), stop=(kt == KT - 1))


def _attention(nc, pools, z, F, nh, hd, wk, wqt, bqk, wv, vis_kv, cc, ind_sb):
    """Full attention for one layer. Returns attn output tiles (fm, bf16).

    Phase-batched: per head-group (one 128-row feature tile = hpt heads),
    quadrant-packed score MMs into a [128, hpt*T<=512] PSUM tile, one exp
    per (group, kt), AV with ones-in-V denominators, then a batched
    ln/exp reciprocal of all denominators and an indicator-matrix
    broadcast multiply.
    """
    sbuf = pools["sbuf"]
    ps_mm, ps_exp, ps_av = pools["ps_mm"], pools["ps_exp"], pools["ps_av"]
    ones_row = pools["ones_row"]
    Dm = F * 128
    KT = L // 128
    hpt = 128 // hd               # heads per 128-row tile (2 enc, 4 dec)
    scale = 1.0 / np.sqrt(hd)
    dram = cc["dram"]

    # --- K feature-major [Dm, T] -> cc-in; AllGather (issued early) ---
    k_cc_in = dram.tile([F, 128, T], BF16, tag="k_cc_in", name="k_cc_in")
    k_cc_out = dram.tile([2, F, 128, T], BF16, tag="k_cc_out", name="k_cc_out")
    for m in range(F):
        ps = ps_mm.tile([128, T], F32, tag="mm", name="mm")
        for k in range(F):
            nc.tensor.matmul(ps, wk[:, k, 128 * m:128 * (m + 1)],
                             z[k], start=(k == 0), stop=(k == F - 1))
        kl = pools["kq"].tile([128, T], BF16, tag=f"kl{m}", name=f"kl{m}")
        nc.scalar.activation(kl, ps, AF.Identity, bias=bqk[:, F + m:F + m + 1])
        nc.sync.dma_start(k_cc_in[m], kl)
    if NO_CC:
        nc.sync.dma_start(k_cc_out[0], k_cc_in[:])
        nc.sync.dma_start(k_cc_out[1], k_cc_in[:])
    else:
        nc.gpsimd.collective_compute(
            "AllGather", ALU.bypass, replica_groups=GROUPS,
            ins=[k_cc_in[:].opt()], outs=[k_cc_out[:].opt()])

    # --- V token-major [T, Dm] -> cc-in; AllGather ---
    v_cc_in = dram.tile([2, 128, Dm], BF16, tag="v_cc_in", name="v_cc_in")
    v_cc_out = dram.tile([2, 2, 128, Dm], BF16, tag="v_cc_out", name="v_cc_out")
    NV = min(Dm // 2, 512)
    ones_nh = pools["ones_nh"]
    for t in range(2):
        vl = pools["v"].tile([128, nh, hd + 1], BF16, tag=f"vl{t}", name=f"vl{t}")
        if vis_kv is None:
            nc.vector.memset(vl[:, :, hd:hd + 1], 1.0)
        else:
            nc.vector.tensor_scalar_mul(vl[:, :, hd:hd + 1],
                                        ones_nh[:, 0:nh, :],
                                        vis_kv[0][:, t:t + 1])
        hpn = NV // hd            # heads per NV block
        for n in range(Dm // NV):
            ps = ps_mm.tile([128, NV], F32, tag="mm", name="mm")
            for k in range(F):
                nc.tensor.matmul(ps, z[k][:, 128 * t:128 * (t + 1)],
                                 wv[:, k, NV * n:NV * (n + 1)],
                                 start=(k == 0), stop=(k == F - 1))
            if vis_kv is None:
                nc.vector.tensor_copy(
                    vl[:, hpn * n:hpn * (n + 1), 0:hd],
                    ps.rearrange("p (h d) -> p h d", h=hpn))
            else:
                nc.vector.tensor_scalar_mul(
                    vl[:, hpn * n:hpn * (n + 1), 0:hd],
                    ps.rearrange("p (h d) -> p h d", h=hpn),
                    vis_kv[0][:, t:t + 1])
        nc.sync.dma_start(
            v_cc_in[t].rearrange("p (h d) -> p h d", h=nh),
            vl[:, :, 0:hd])
    if NO_CC:
        nc.sync.dma_start(v_cc_out[0], v_cc_in[:])
        nc.sync.dma_start(v_cc_out[1], v_cc_in[:])
    else:
        nc.gpsimd.collective_compute(
            "AllGather", ALU.bypass, replica_groups=GROUPS,
            ins=[v_cc_in[:].opt()], outs=[v_cc_out[:].opt()])

    # --- Q feature-major (overlaps the collectives) ---
    q_sb = []
    for m in range(F):
        ps = ps_mm.tile([128, T], F32, tag="mm", name="mm")
        for k in range(F):
            nc.tensor.matmul(ps, wqt[:, k, 128 * m:128 * (m + 1)],
                             z[k], start=(k == 0), stop=(k == F - 1))
        q = pools["kq"].tile([128, T], BF16, tag=f"q{m}", name=f"q{m}")
        nc.scalar.activation(q, ps, AF.Identity, bias=bqk[:, m:m + 1])
        q_sb.append(q)

    # --- readback K_full [F][128, L] and V_full [KT][128, nh, hd+1] ---
    k_full = []
    for m in range(F):
        kf = pools["kf"].tile([128, L], BF16, tag=f"kf{m}", name=f"kf{m}")
        nc.sync.dma_start(kf[:, 0:T], k_cc_out[0, m])
        nc.sync.dma_start(kf[:, T:L], k_cc_out[1, m])
        k_full.append(kf)
    v_full = []
    for kt in range(KT):
        vf = pools["v"].tile([128, nh, hd + 1], BF16, tag=f"vf{kt}",
                             name=f"vf{kt}")
        if vis_kv is None:
            nc.vector.memset(vf[:, :, hd:hd + 1], 1.0)
        else:
            nc.vector.tensor_scalar_mul(vf[:, :, hd:hd + 1],
                                        ones_nh[:, 0:nh, :],
                                        vis_kv[1][:, kt:kt + 1])
        nc.sync.dma_start(
            vf[:, :, 0:hd],
            v_cc_out[kt // 2, kt % 2].rearrange("p (h d) -> p h d", h=nh))
        v_full.append(vf)

    if os.environ.get("BISECT_ATTN_QKV"):
        return q_sb
    # --- phase-batched scores / exp / AV ---
    # denominator grid [128, ceil(nh/4)*T]: head h at partition 32*(h%4),
    # columns (h//4)*T onward -- every engine AP stays 32-aligned.
    nhb = (nh + 3) // 4
    noden = bool(os.environ.get("BISECT_ATTN_NODEN"))
    den4 = pools["dn"].tile([128, nhb * T], F32, tag="den4", name="den4")
    if not noden:
        nc.vector.memset(den4, 1.0)
    uv_tiles = []
    for g in range(F):
        uv = pools["uv"].tile([128, T], BF16, tag=f"uv{g}", name=f"uv{g}")
        for j in range(hpt):
            h = g * hpt + j
            ro = j * hd
            # one av tile (= one PSUM bank) per head: single accumulation
            # group per bank; score kt-pairs share a bank but issue from
            # the same PE quadrant, so their drains are serialized
            av = ps_av.tile([128, T], F32, tag="av", name="av")
            a_tiles = {}
            for kp in range(KT // 2):
                s = ps_exp.tile([128, 2 * T], F32, tag="exp", name="exp")
                for half in range(2):
                    kt = 2 * kp + half
                    nc.tensor.matmul(
                        s[:, T * half:T * (half + 1)],
                        k_full[g][ro:ro + hd, 128 * kt:128 * (kt + 1)],
                        q_sb[g][ro:ro + hd, :], start=True, stop=True,
                        tile_position=(ro, 0))
                a_sb = pools["a"].tile([128, 2 * T], BF16, tag="a", name="a")
                nc.scalar.activation(a_sb, s, AF.Exp, scale=scale)
                a_tiles[kp] = a_sb
                if kp >= 1:
                    _attn_av(nc, a_tiles, v_full, av, h, hd, kp - 1, KT)
            _attn_av(nc, a_tiles, v_full, av, h, hd, KT // 2 - 1, KT)
            # drain AV: uv (attn rows, bf16) + ln(den) into the aligned grid
            nc.scalar.activation(uv[j * hd:(j + 1) * hd, :],
                                 av[0:hd, :], AF.Identity)
            if not noden:
                r0 = 32 * (h % 4)
                nc.scalar.activation(
                    den4[r0:r0 + 1, (h // 4) * T:(h // 4 + 1) * T],
                    av[hd:hd + 1, :], AF.Identity)
        uv_tiles.append(uv)

    if os.environ.get("BISECT_ATTN_NONORM"):
        return uv_tiles
    # batched 1/den over the whole grid (approx reciprocal on VectorE)
    nc.vector.reciprocal_approx_fast(den4, den4)
    rcb = pools["dn"].tile([128, nhb * T], BF16, tag="rcb", name="rcb")
    nc.vector.tensor_copy(rcb, den4)

    # broadcast 1/den to head rows via aligned K=1 outer products; normalize
    ones_sq = pools["ones_sq"]
    attn = []
    for g in range(F):
        bca = pools["ps_bc"].tile([128, T], F32, tag="bc", name="bca")
        for j in range(hpt):
            h = g * hpt + j
            r0 = 32 * (h % 4)
            nc.tensor.matmul(bca[j * hd:(j + 1) * hd, :],
                             ones_sq[r0:r0 + 1, 0:hd],
                             rcb[r0:r0 + 1, (h // 4) * T:(h // 4 + 1) * T],
                             start=True, stop=True,
                             tile_position=(r0, j * hd))
        a = sbuf.tile([128, T], BF16, tag=f"attn{g}", name=f"attn{g}")
        nc.vector.tensor_mul(a, uv_tiles[g], bca)
        attn.append(a)
    return attn


def _layer(nc, pools, cc, x_tiles, F, n_heads, hd, waps, vis_kv, ind_sb,
           wpools):
    """One transformer block (attn + MLP) updating x_tiles in place."""
    wq_pool, wv_pool, wp_pool, w1_pool, w2_pool, bias_pool = wpools
    (a_wqk, a_bqk, a_wv, a_wpr, a_bpr, a_wf1, a_bf1w, a_wf2, a_bf2) = waps
    ps_mm = pools["ps_mm"]
    ones_row = pools["ones_row"]
    Dm = F * 128
    F1 = a_wf1.shape[-1] // 128   # hidden tiles (24 enc / 16 dec)

    wk = wq_pool.tile([128, F, Dm], BF16, tag="wk", name="wk")
    nc.gpsimd.dma_start(wk, a_wqk[:, :, Dm:2 * Dm])
    wqt = wv_pool.tile([128, F, Dm], BF16, tag="wqt", name="wqt")
    nc.gpsimd.dma_start(wqt, a_wqk[:, :, 0:Dm])
    wv = wv_pool.tile([128, F, Dm], BF16, tag="wv", name="wv")
    nc.gpsimd.dma_start(wv, a_wv)
    wpr = wp_pool.tile([128, F, Dm], BF16, tag="wpr", name="wpr")
    nc.gpsimd.dma_start(wpr, a_wpr)
    bqk = bias_pool.tile([128, 2 * F], F32, tag="bqk", name="bqk")
    nc.sync.dma_start(bqk, a_bqk)
    bf1w = bias_pool.tile([1, F1 * 128], BF16, tag="bf1w", name="bf1w")
    nc.sync.dma_start(bf1w, a_bf1w)
    bpr = bias_pool.tile([128, F], F32, tag="bpr", name="bpr")
    nc.sync.dma_start(bpr, a_bpr)
    bf2 = bias_pool.tile([128, F], F32, tag="bf2", name="bf2")
    nc.sync.dma_start(bf2, a_bf2)

    z = _ln_to_z(nc, pools, x_tiles, F)
    if os.environ.get("BISECT_SKIP_ATTN"):
        attn = z
    else:
        attn = _attention(nc, pools, z, F, n_heads, hd, wk, wqt, bqk, wv,
                          vis_kv, cc, ind_sb)
    for m in range(F):
        ps = ps_mm.tile([128, T], F32, tag="mm", name="mm")
        for k in range(F):
            nc.tensor.matmul(ps, wpr[:, k, 128 * m:128 * (m + 1)],
                             attn[k], start=(k == 0), stop=(k == F - 1))
        nc.vector.scalar_tensor_tensor(x_tiles[m], ps, bpr[:, m:m + 1],
                                       x_tiles[m], ALU.add, ALU.add)
    if os.environ.get("BISECT_SKIP_MLP"):
        return
    z2 = _ln_to_z(nc, pools, x_tiles, F)

    # f1 in pairs of m-tiles sharing a [128, 512] PSUM bank; bias folded
    # as a K=1 matmul so one GELU covers both halves
    hmid = []
    mpb = F1 // 4
    for b in range(4):
        w1b = w1_pool.tile([128, F, 128 * mpb], BF16, tag="wf1", name="wf1")
        nc.gpsimd.dma_start(w1b, a_wf1[:, :, 128 * mpb * b:128 * mpb * (b + 1)])
        for mp in range(mpb // 2):
            ps = ps_mm.tile([128, 2 * T], F32, tag="mm", name="mm")
            for half in range(2):
                m = 2 * mp + half
                mg = b * mpb + m
                for k in range(F):
                    nc.tensor.matmul(ps[:, T * half:T * (half + 1)],
                                     w1b[:, k, 128 * m:128 * (m + 1)],
                                     z2[k], start=(k == 0), stop=False)
                nc.tensor.matmul(ps[:, T * half:T * (half + 1)],
                                 bf1w[0:1, 128 * mg:128 * (mg + 1)],
                                 ones_row[0:1, :], start=False, stop=True)
            hm = pools["h"].tile([128, 2 * T], BF16, tag=f"hm{b * mpb // 2 + mp}",
                                 name=f"hm{mg}")
            nc.scalar.activation(hm, ps, AF.Gelu)
            hmid.append(hm)
    # f2 streamed in column-blocks of 2 m-tiles
    for b in range(F // 2):
        w2b = w2_pool.tile([128, F1, 256], BF16, tag="wf2", name="wf2")
        nc.gpsimd.dma_start(w2b, a_wf2[:, :, 256 * b:256 * (b + 1)])
        for mm in range(2):
            m = 2 * b + mm
            ps = ps_mm.tile([128, T], F32, tag="mm", name="mm")
            for k in range(F1 // 2):
                for half in range(2):
                    nc.tensor.matmul(ps, w2b[:, 2 * k + half, 128 * mm:128 * (mm + 1)],
                                     hmid[k][:, T * half:T * (half + 1)],
                                     start=(k == 0 and half == 0),
                                     stop=(k == F1 // 2 - 1 and half == 1))
            nc.vector.scalar_tensor_tensor(x_tiles[m], ps, bf2[:, m:m + 1],
                                           x_tiles[m], ALU.add, ALU.add)


def build_program(n_enc=N_ENC, n_dec=N_DEC, no_cc=False):
    global NO_CC
    NO_CC = no_cc
    nc = bacc.Bacc("TRN2", target_bir_lowering=False, debug=False, num_devices=8)

    def inp(name, shape, dt=BF16):
        return nc.dram_tensor(name, shape, dt, kind="ExternalInput").ap()

    # --- inputs (per-core) ---
    patches_t = inp("patches_t", [128, 8, T])
    posf_t = inp("posf_t", [5, T])
    w_pe = inp("w_pe", [128, 8, D])
    b_embed = inp("b_embed", [128, 6], F32)
    w_pos1 = inp("w_pos1", [5, 384])
    b_pos1 = inp("b_pos1", [128, 3], F32)
    w_pos2 = inp("w_pos2", [128, 3, D])
    if n_enc:
        e_wqk = inp("e_wqk", [n_enc, 128, 6, 1536])
        e_bqk = inp("e_bqk", [n_enc, 128, 12], F32)
        e_wv = inp("e_wv", [n_enc, 128, 6, D])
        e_wpr = inp("e_wpr", [n_enc, 128, 6, D])
        e_bpr = inp("e_bpr", [n_enc, 128, 6], F32)
        e_wf1 = inp("e_wf1", [n_enc, 128, 6, 3072])
        e_bf1w = inp("e_bf1w", [n_enc, 1, 3072])
        e_wf2 = inp("e_wf2", [n_enc, 128, 24, D])
        e_bf2 = inp("e_bf2", [n_enc, 128, 6], F32)
    vis_loc = inp("vis_loc", [128, 2], F32)
    vis_glob = inp("vis_glob", [128, 4], F32)
    vis = inp("vis", [128, T], F32)
    enw = inp("enw", [128, 6], F32)
    enb = inp("enb", [128, 6], F32)
    w_de = inp("w_de", [128, 6, DD])
    b_de = inp("b_de", [128, 4], F32)
    mtk = inp("mtk", [128, 4, T])
    w_dpos1 = inp("w_dpos1", [5, 256])
    b_dpos1 = inp("b_dpos1", [128, 2], F32)
    w_dpos2 = inp("w_dpos2", [128, 2, DD])
    b_dpos2 = inp("b_dpos2", [128, 4], F32)
    if n_dec:
        d_wqk = inp("d_wqk", [n_dec, 128, 4, 1024])
        d_bqk = inp("d_bqk", [n_dec, 128, 8], F32)
        d_wv = inp("d_wv", [n_dec, 128, 4, DD])
        d_wpr = inp("d_wpr", [n_dec, 128, 4, DD])
        d_bpr = inp("d_bpr", [n_dec, 128, 4], F32)
        d_wf1 = inp("d_wf1", [n_dec, 128, 4, 2048])
        d_bf1w = inp("d_bf1w", [n_dec, 1, 2048])
        d_wf2 = inp("d_wf2", [n_dec, 128, 16, DD])
        d_bf2 = inp("d_bf2", [n_dec, 128, 4], F32)
    w_hi = inp("w_hi", [128, 4, PD])
    w_hn = inp("w_hn", [128, 4, PD])
    b_hi = inp("b_hi", [1, PD], F32)
    b_hn = inp("b_hn", [1, PD], F32)
    ind_e_in = inp("ind_e", [2, 128])
    ind_d_in = inp("ind_d", [4, 128])
    out_i = nc.dram_tensor("out_i", [T, PD], F32, kind="ExternalOutput").ap()
    out_n = nc.dram_tensor("out_n", [T, PD], F32, kind="ExternalOutput").ap()

    from contextlib import ExitStack
    with tile.TileContext(nc) as tc, ExitStack() as es:
        sbuf = es.enter_context(tc.tile_pool(name="sbuf", bufs=1))
        consts = es.enter_context(tc.tile_pool(name="consts", bufs=1))
        xpool = es.enter_context(tc.tile_pool(name="x", bufs=1))
        zpool = es.enter_context(tc.tile_pool(name="z", bufs=2))
        sqpool = es.enter_context(tc.tile_pool(name="sq", bufs=3))
        kqpool = es.enter_context(tc.tile_pool(name="kq", bufs=1))
        kfpool = es.enter_context(tc.tile_pool(name="kf", bufs=1))
        vpool = es.enter_context(tc.tile_pool(name="v", bufs=1))
        apool = es.enter_context(tc.tile_pool(name="a", bufs=4))
        uvpool = es.enter_context(tc.tile_pool(name="uv", bufs=1))
        dnpool = es.enter_context(tc.tile_pool(name="dn", bufs=1))
        hpool = es.enter_context(tc.tile_pool(name="h", bufs=1))
        wq_pool = es.enter_context(tc.tile_pool(name="wq", bufs=2))
        wv_pool = es.enter_context(tc.tile_pool(name="wv", bufs=1))
        wp_pool = es.enter_context(tc.tile_pool(name="wp", bufs=1))
        w1_pool = es.enter_context(tc.tile_pool(name="w1", bufs=2))
        w2_pool = es.enter_context(tc.tile_pool(name="w2", bufs=2))
        bias_pool = es.enter_context(tc.tile_pool(name="bias", bufs=1))
        ps_mm = es.enter_context(tc.tile_pool(name="ps_mm", bufs=2, space="PSUM"))
        ps_exp = es.enter_context(tc.tile_pool(name="ps_exp", bufs=2, space="PSUM"))
        ps_av = es.enter_context(tc.tile_pool(name="ps_av", bufs=2, space="PSUM"))
        ps_st = es.enter_context(tc.tile_pool(name="ps_st", bufs=1, space="PSUM"))
        ps_bc = es.enter_context(tc.tile_pool(name="ps_bc", bufs=1, space="PSUM"))
        dram = es.enter_context(tc.tile_pool(name="dram", bufs=2, space="DRAM"))

        pools = dict(sbuf=sbuf, z=zpool, sq=sqpool, kq=kqpool, kf=kfpool,
                     v=vpool, a=apool, uv=uvpool, dn=dnpool, h=hpool,
                     ps_mm=ps_mm, ps_exp=ps_exp, ps_av=ps_av, ps_st=ps_st,
                     ps_bc=ps_bc)
        cc = dict(dram=dram)
        wpools = (wq_pool, wv_pool, wp_pool, w1_pool, w2_pool, bias_pool)

        ones_col = consts.tile([128, 1], BF16)
        nc.vector.memset(ones_col, 1.0)
        ones_row = consts.tile([1, T], BF16)
        nc.vector.memset(ones_row, 1.0)
        ones_bf = consts.tile([1, 128], BF16)
        nc.vector.memset(ones_bf, 1.0)
        eps_sb = consts.tile([1, 1], F32)
        nc.vector.memset(eps_sb, EPS)
        pools["eps"] = eps_sb
        pools["ones_col"] = ones_col
        pools["ones_row"] = ones_row
        pools["ones_bf"] = ones_bf

        vis_loc_sb = consts.tile([128, 2], F32)
        nc.sync.dma_start(vis_loc_sb, vis_loc)
        vis_glob_sb = consts.tile([128, 4], F32)
        nc.sync.dma_start(vis_glob_sb, vis_glob)
        ones_nh = consts.tile([128, 16, 1], BF16)
        nc.vector.memset(ones_nh, 1.0)
        pools["ones_nh"] = ones_nh
        vis_kv = (vis_loc_sb, vis_glob_sb)
        vis_sb = consts.tile([128, T], F32)
        nc.sync.dma_start(vis_sb, vis)
        pf = consts.tile([5, T], BF16)
        nc.sync.dma_start(pf, posf_t)
        ones_sq = consts.tile([128, 128], BF16)
        nc.vector.memset(ones_sq, 1.0)
        pools["ones_sq"] = ones_sq

        # ===== embedding (scoped pool, released after) =====
        embed_pool = tc.alloc_tile_pool(name="embed", bufs=1)
        wp1 = embed_pool.tile([5, 384], BF16)
        nc.sync.dma_start(wp1, w_pos1)
        wp2 = embed_pool.tile([128, 3, D], BF16)
        nc.sync.dma_start(wp2, w_pos2)
        bp1 = embed_pool.tile([128, 3], F32)
        nc.sync.dma_start(bp1, b_pos1)
        pt = embed_pool.tile([128, 8, T], BF16)
        nc.sync.dma_start(pt, patches_t)
        wpe = embed_pool.tile([128, 8, D], BF16)
        nc.gpsimd.dma_start(wpe, w_pe)
        bemb = embed_pool.tile([128, 6], F32)
        nc.sync.dma_start(bemb, b_embed)

        h1 = []
        for m in range(3):
            ps = ps_mm.tile([128, T], F32, tag="mm", name="mm")
            nc.tensor.matmul(ps, wp1[:, 128 * m:128 * (m + 1)], pf,
                             start=True, stop=True)
            t = embed_pool.tile([128, T], BF16, tag=f"h1_{m}", name=f"h1_{m}")
            nc.scalar.activation(t, ps, AF.Gelu, bias=bp1[:, m:m + 1])
            h1.append(t)

        x_tiles = [xpool.tile([128, T], F32, tag=f"x{k}", name=f"x{k}")
                   for k in range(6)]
        for m in range(6):
            ps = ps_mm.tile([128, T], F32, tag="mm", name="mm")
            for k in range(8):
                nc.tensor.matmul(ps, wpe[:, k, 128 * m:128 * (m + 1)],
                                 pt[:, k, :], start=(k == 0), stop=False)
            for k in range(3):
                nc.tensor.matmul(ps, wp2[:, k, 128 * m:128 * (m + 1)],
                                 h1[k], start=False, stop=(k == 2))
            nc.scalar.activation(x_tiles[m], ps, AF.Identity, bias=bemb[:, m:m + 1])
        embed_pool.release()

        # ===== encoder =====
        for i in range(n_enc):
            waps = (e_wqk[i], e_bqk[i], e_wv[i], e_wpr[i], e_bpr[i],
                    e_wf1[i], e_bf1w[i], e_wf2[i], e_bf2[i])
            _layer(nc, pools, cc, x_tiles, 6, ENC_H, ENC_HD, waps,
                   vis_kv, None, wpools)

        # ===== bridge: enc norm + mask + decoder embed (scoped pool) =====
        bridge = tc.alloc_tile_pool(name="bridge", bufs=1)
        enw_sb = bridge.tile([128, 6], F32)
        nc.sync.dma_start(enw_sb, enw)
        enb_sb = bridge.tile([128, 6], F32)
        nc.sync.dma_start(enb_sb, enb)
        ze = _ln_to_z(nc, pools, x_tiles, 6, out_pool=bridge, out_tag="ze")
        enc_sb = []
        for k in range(6):
            t = bridge.tile([128, T], F32, tag=f"enc_t{k}", name=f"enc_t{k}")
            nc.scalar.activation(t, ze[k], AF.Identity, bias=enb_sb[:, k:k + 1],
                                 scale=enw_sb[:, k:k + 1])
            e = bridge.tile([128, T], BF16, tag=f"enc{k}", name=f"enc{k}")
            nc.vector.tensor_mul(e, t, vis_sb)
            enc_sb.append(e)

        wde = bridge.tile([128, 6, DD], BF16)
        nc.gpsimd.dma_start(wde, w_de)
        bde = bridge.tile([128, 4], F32)
        nc.sync.dma_start(bde, b_de)
        mtk_sb = bridge.tile([128, 4, T], BF16)
        nc.sync.dma_start(mtk_sb, mtk)
        wd1 = bridge.tile([5, 256], BF16)
        nc.sync.dma_start(wd1, w_dpos1)
        bd1 = bridge.tile([128, 2], F32)
        nc.sync.dma_start(bd1, b_dpos1)
        wd2 = bridge.tile([128, 2, DD], BF16)
        nc.sync.dma_start(wd2, w_dpos2)
        bd2 = bridge.tile([128, 4], F32)
        nc.sync.dma_start(bd2, b_dpos2)

        h1d = []
        for m in range(2):
            ps = ps_mm.tile([128, T], F32, tag="mm", name="mm")
            nc.tensor.matmul(ps, wd1[:, 128 * m:128 * (m + 1)], pf,
                             start=True, stop=True)
            t = bridge.tile([128, T], BF16, tag=f"h1d_{m}", name=f"h1d_{m}")
            nc.scalar.activation(t, ps, AF.Gelu, bias=bd1[:, m:m + 1])
            h1d.append(t)

        xd_tiles = [xpool.tile([128, T], F32, tag=f"xd{k}", name=f"xd{k}")
                    for k in range(4)]
        for m in range(4):
            ps = ps_mm.tile([128, T], F32, tag="mm", name="mm")
            for k in range(6):
                nc.tensor.matmul(ps, wde[:, k, 128 * m:128 * (m + 1)],
                                 enc_sb[k], start=(k == 0), stop=(k == 5))
            t1 = bridge.tile([128, T], F32, tag="dec_t1", name="dec_t1")
            nc.scalar.activation(t1, ps, AF.Identity, bias=bde[:, m:m + 1])
            nc.vector.tensor_mul(t1, t1, vis_sb)
            nc.vector.tensor_add(t1, t1, mtk_sb[:, m, :])
            ps2 = ps_mm.tile([128, T], F32, tag="mm", name="mm")
            for k in range(2):
                nc.tensor.matmul(ps2, wd2[:, k, 128 * m:128 * (m + 1)],
                                 h1d[k], start=(k == 0), stop=(k == 1))
            t2 = bridge.tile([128, T], F32, tag="dec_t2", name="dec_t2")
            nc.scalar.activation(t2, ps2, AF.Identity, bias=bd2[:, m:m + 1])
            nc.vector.tensor_add(xd_tiles[m], t1, t2)
        bridge.release()

        # ===== decoder =====
        for i in range(n_dec):
            waps = (d_wqk[i], d_bqk[i], d_wv[i], d_wpr[i], d_bpr[i],
                    d_wf1[i], d_bf1w[i], d_wf2[i], d_bf2[i])
            _layer(nc, pools, cc, xd_tiles, 4, DEC_H, DEC_HD, waps,
                   None, None, wpools)

        # ===== final norm + heads (token-major output) =====
        tail = tc.alloc_tile_pool(name="tail", bufs=2)
        zf = _ln_to_z(nc, pools, xd_tiles, 4)
        for (a_wh, a_bh, outdram) in ((w_hi, b_hi, out_i), (w_hn, b_hn, out_n)):
            wh = tail.tile([128, 4, PD], BF16, tag="wh", name="wh")
            nc.gpsimd.dma_start(wh, a_wh)
            bh = tail.tile([128, PD], F32, tag="bh", name="bh")
            nc.sync.dma_start(bh, a_bh.to_broadcast([128, PD]))
            for t in range(2):
                for n in range(2):
                    ps = ps_exp.tile([128, 512], F32, tag="exp", name="head_ps")
                    for k in range(4):
                        nc.tensor.matmul(ps, zf[k][:, 128 * t:128 * (t + 1)],
                                         wh[:, k, 512 * n:512 * (n + 1)],
                                         start=(k == 0), stop=(k == 3))
                    o = tail.tile([128, 512], F32, tag="headout", name="headout")
                    nc.vector.tensor_add(o, ps, bh[:, 512 * n:512 * (n + 1)])
                    nc.sync.dma_start(
                        outdram[128 * t:128 * (t + 1), 512 * n:512 * (n + 1)], o)
        tail.release()

    nc.compile()
    return nc


# ---------------- host side ----------------

def _chunk_w(w, dtype=NBF):
    """[Din, Dout] -> [128, Din//128, Dout]"""
    din, dout = w.shape
    return np.ascontiguousarray(
        w.reshape(din // 128, 128, dout).transpose(1, 0, 2)).astype(dtype)


def _chunk_b(b, dtype=np.float32):
    """[Dout] -> [128, Dout//128] column-chunk layout"""
    return np.ascontiguousarray(b.reshape(-1, 128).T).astype(dtype)


def prep_inputs(inputs, n_enc=N_ENC, n_dec=N_DEC):
    f32 = np.float32
    g = {k: np.asarray(v, f32) if np.asarray(v).dtype != np.int32 else np.asarray(v)
         for k, v in inputs.items()}
    IMG, MAXD = 1024.0, 8.0
    coords, depths, mask = g["coords"], g["depths"], g["mask"]
    x1, x2, y1, y2 = coords[..., 0], coords[..., 1], coords[..., 2], coords[..., 3]
    posf = np.stack([(x1 + x2) / 2.0 / IMG, (y1 + y2) / 2.0 / IMG,
                     (x2 - x1) / IMG, (y2 - y1) / IMG, depths / MAXD], -1)
    patches = g["patches"].reshape(B, L, PD)
    visible = (mask == 0).astype(f32)  # [B, L]

    shared = {}
    shared["w_pe"] = _chunk_w(g["pe_w"])
    shared["b_embed"] = _chunk_b(g["pe_b"] + g["pos2_b"])
    shared["w_pos1"] = g["pos1_w"].astype(NBF)
    shared["b_pos1"] = _chunk_b(g["pos1_b"])
    shared["w_pos2"] = _chunk_w(g["pos2_w"])

    def layer_stack(n, lnw1, lnb1, qkvw, qkvb, prw, prb, lnw2, lnb2,
                    f1w, f1b, f2w, f2b, d_model):
        o = {k: [] for k in ("wqk", "bqk", "wv", "wpr", "bpr", "wf1", "bf1w",
                             "wf2", "bf2")}
        if n == 0:
            return {}
        for i in range(n):
            w_qk = lnw1[i][:, None] * qkvw[i][:, :2 * d_model]
            b_qk = lnb1[i] @ qkvw[i][:, :2 * d_model] + qkvb[i][:2 * d_model]
            w_v = lnw1[i][:, None] * qkvw[i][:, 2 * d_model:]
            b_v = lnb1[i] @ qkvw[i][:, 2 * d_model:] + qkvb[i][2 * d_model:]
            o["wqk"].append(_chunk_w(w_qk))
            o["bqk"].append(_chunk_b(b_qk))
            o["wv"].append(_chunk_w(w_v))
            o["wpr"].append(_chunk_w(prw[i]))
            o["bpr"].append(_chunk_b(prb[i] + b_v @ prw[i]))
            w_f1 = lnw2[i][:, None] * f1w[i]
            b_f1 = lnb2[i] @ f1w[i] + f1b[i]
            o["wf1"].append(_chunk_w(w_f1))
            o["bf1w"].append(b_f1.astype(NBF)[None, :])
            o["wf2"].append(_chunk_w(f2w[i]))
            o["bf2"].append(_chunk_b(f2b[i]))
        return {k: np.stack(v) for k, v in o.items()}

    enc = layer_stack(n_enc, g["e_ln1_w"], g["e_ln1_b"], g["e_qkv_w"], g["e_qkv_b"],
                      g["e_pr_w"], g["e_pr_b"], g["e_ln2_w"], g["e_ln2_b"],
                      g["e_f1_w"], g["e_f1_b"], g["e_f2_w"], g["e_f2_b"], D)
    dec = layer_stack(n_dec, g["d_ln1_w"], g["d_ln1_b"], g["d_qkv_w"], g["d_qkv_b"],
                      g["d_pr_w"], g["d_pr_b"], g["d_ln2_w"], g["d_ln2_b"],
                      g["d_f1_w"], g["d_f1_b"], g["d_f2_w"], g["d_f2_b"], DD)
    for k, v in enc.items():
        shared[f"e_{k}"] = v
    for k, v in dec.items():
        shared[f"d_{k}"] = v

    shared["enw"] = _chunk_b(g["enorm_w"])
    shared["enb"] = _chunk_b(g["enorm_b"])
    shared["w_de"] = _chunk_w(g["de_w"])
    shared["b_de"] = _chunk_b(g["de_b"])
    shared["w_dpos1"] = g["dpos1_w"].astype(NBF)
    shared["b_dpos1"] = _chunk_b(g["dpos1_b"])
    shared["w_dpos2"] = _chunk_w(g["dpos2_w"])
    shared["b_dpos2"] = _chunk_b(g["dpos2_b"])
    shared["w_hi"] = _chunk_w(g["dnorm_w"][:, None] * g["hi_w"])
    shared["b_hi"] = (g["dnorm_b"] @ g["hi_w"] + g["hi_b"]).astype(f32)[None, :]
    shared["w_hn"] = _chunk_w(g["dnorm_w"][:, None] * g["hn_w"])
    shared["b_hn"] = (g["dnorm_b"] @ g["hn_w"] + g["hn_b"]).astype(f32)[None, :]
    ind_e_np = np.zeros((2, 128), NBF)
    for j in range(2):
        ind_e_np[j, 64 * j:64 * (j + 1)] = 1
    shared["ind_e"] = ind_e_np
    ind_d_np = np.zeros((4, 128), NBF)
    for j in range(4):
        ind_d_np[j, 32 * j:32 * (j + 1)] = 1
    shared["ind_d"] = ind_d_np

    in_maps = []
    for c in range(8):
        b, h = c // 2, c % 2
        sl = slice(h * T, (h + 1) * T)
        m = dict(shared)
        m["patches_t"] = np.ascontiguousarray(
            patches[b, sl].T.reshape(8, 128, T).transpose(1, 0, 2)).astype(NBF)
        m["posf_t"] = np.ascontiguousarray(posf[b, sl].T).astype(NBF)
        vb = visible[b]
        m["vis_glob"] = np.ascontiguousarray(vb.astype(f32).reshape(4, 128).T)
        m["vis_loc"] = np.ascontiguousarray(
            visible[b, sl].astype(f32).reshape(2, 128).T)
        vloc = visible[b, sl]
        m["vis"] = np.broadcast_to(vloc[None, :], (128, T)).astype(f32).copy()
        m["mtk"] = np.ascontiguousarray(
            (g["mask_token"].reshape(4, 128)[:, :, None] *
             (1.0 - vloc)[None, None, :]).transpose(1, 0, 2)).astype(NBF)
        in_maps.append(m)
    return in_maps


_PROG = {}


def _get_prog(n_enc=N_ENC, n_dec=N_DEC):
    key = (n_enc, n_dec)
    if key not in _PROG:
        _PROG[key] = build_program(n_enc, n_dec)
    return _PROG[key]


def run(inputs, n_enc=N_ENC, n_dec=N_DEC, **kwargs):
    nc = _get_prog(n_enc, n_dec)
    in_maps = prep_inputs(inputs, n_enc, n_dec)
    res = run_bass_kernel_spmd(nc, in_maps, core_ids=list(range(8)), **kwargs)
    oi = np.zeros((B, L, PD), np.float32)
    on = np.zeros((B, L, PD), np.float32)
    for c in range(8):
        b, h = c // 2, c % 2
        oi[b, h * T:(h + 1) * T] = res.results[c]["out_i"]
        on[b, h * T:(h + 1) * T] = res.results[c]["out_n"]
    return (oi, on), res


def kernel(**inputs):
    (oi, on), _ = run(inputs)
    return oi, on
